# revision 9
# baseline (speedup 1.0000x reference)
"""Trainium2 Bass kernel for nn_EndToEndCryptoModel (LSTM -> GCNx2 -> Dense).

Strategy (per-core, data-parallel over batch, 4 batches/core on 8 cores):
  * LSTM solved by Picard fixed-point iteration over the whole sequence:
    7 iterations, each fully parallel over (b, t) using one big sigmoid op
    (all 4 gates via sigmoid with per-partition scale), and the cell-state
    recurrence done by a single DVE tensor_tensor_scan along the time axis
    (batch chains separated by poison pad columns that reset the scan).
  * The GCN collapses algebraically: sup1 is node-independent, so
    g1 = leaky(rowsum(a) (x) s1) is rank-1 (leaky is positively homogeneous
    and b1 == 0), and the whole two-layer GCN reduces to per-(t,m) scalars
    q[t,m] and per-node weights w[n] = (a @ (a @ 1))[n].
  * Final dense layer: d1[b,p] = sum_{t,m} Lq'[b,t,m] * (w[b,:] @ D[t,:,m,p]),
    computed as 64 small matmuls with D t-slices as stationary weights into a
    [(m,p)=96, (t,b)] PSUM layout that exactly matches the layout q comes in
    (w2 columns pre-replicated x3 on the host), then DVE multiply+reduce and
    two tiny matmuls.

All heavy layout decisions are hardcoded for the fixed problem shapes.
"""

import numpy as np

B, T, N, F = 32, 64, 128, 128
U, K1, K2 = 64, 64, 32
NCORE = 8
BL = B // NCORE            # 4 batches per core
CW = BL * (T + 1)          # 260 columns, b-major, pad col at b*(T+1)
NEG = -1e30
EPS = 1e-3
SLOPE = 0.01
N_ITERS = 7

_CACHE = {}


def build_module():
    """Build the per-core Bass/Tile module (identical SPMD program)."""
    from contextlib import ExitStack
    import concourse.bass as bass
    import concourse.bacc as bacc
    import concourse.mybir as mybir
    from concourse import tile

    f32 = mybir.dt.float32
    Alu = mybir.AluOpType
    Act = mybir.ActivationFunctionType

    nc = bacc.Bacc(None, target_bir_lowering=False)

    # ---------------- DRAM I/O ----------------
    x_d = nc.dram_tensor("x_sh", [BL * T, F], f32, kind="ExternalInput")
    a_d = nc.dram_tensor("a_sh", [BL, N, N], f32, kind="ExternalInput")
    wk0_d = nc.dram_tensor("wk0", [F, 128], f32, kind="ExternalInput")
    wk1_d = nc.dram_tensor("wk1", [F, 128], f32, kind="ExternalInput")
    wr0_d = nc.dram_tensor("wr0", [U, 128], f32, kind="ExternalInput")
    wr1_d = nc.dram_tensor("wr1", [U, 128], f32, kind="ExternalInput")
    bias0_d = nc.dram_tensor("bias0", [128, 1], f32, kind="ExternalInput")
    bias1_d = nc.dram_tensor("bias1", [128, 1], f32, kind="ExternalInput")
    w1p_d = nc.dram_tensor("w1p", [U, K1], f32, kind="ExternalInput")
    c1_d = nc.dram_tensor("c1", [K1, 1], f32, kind="ExternalInput")
    c1n_d = nc.dram_tensor("c1n", [K1, 1], f32, kind="ExternalInput")
    w2rep_d = nc.dram_tensor("w2rep", [K1, 96], f32, kind="ExternalInput")
    d1w_d = nc.dram_tensor("d1w", [T * N * K2, 3], f32, kind="ExternalInput")
    d1b_d = nc.dram_tensor("d1b3", [3, 1], f32, kind="ExternalInput")
    d2w_d = nc.dram_tensor("d2w", [3, N], f32, kind="ExternalInput")
    d2b_d = nc.dram_tensor("d2b", [1, N], f32, kind="ExternalInput")
    out_d = nc.dram_tensor("out_sh", [BL, N], f32, kind="ExternalOutput")

    # ---------------- structural constants (baked into NEFF) ----------------
    ident_d = nc.inline_tensor(np.eye(128, dtype=np.float32), "ident")
    scale1_np = np.concatenate(
        [2 * np.ones(U, np.float32), np.ones(U, np.float32)]
    ).reshape(128, 1)
    scale1_d = nc.inline_tensor(scale1_np, "scale1")
    sel_np = np.zeros((96, 3), np.float32)
    for mm_ in range(K2):
        for pp in range(3):
            sel_np[mm_ * 3 + pp, pp] = 1.0
    sel96_d = nc.inline_tensor(sel_np, "sel96")
    ones128_d = nc.inline_tensor(np.ones((128, 1), np.float32), "ones128")
    ones14_d = nc.inline_tensor(np.ones((1, BL), np.float32), "ones14")

    with tile.TileContext(nc) as tc, ExitStack() as ctx:
        cp = ctx.enter_context(tc.tile_pool(name="const", bufs=1))
        wp = ctx.enter_context(tc.tile_pool(name="work", bufs=2))
        pz = ctx.enter_context(tc.tile_pool(name="pz", bufs=1, space="PSUM"))
        pm = ctx.enter_context(tc.tile_pool(name="pm", bufs=2, space="PSUM"))
        pt = ctx.enter_context(tc.tile_pool(name="pt", bufs=2, space="PSUM"))
        ps = ctx.enter_context(tc.tile_pool(name="ps", bufs=2, space="PSUM"))

        dma = nc.sync.dma_start

        # ---- big D prefetch first: [n=128, (t, m, p) = 6144] ----
        D_sb = cp.tile([128, T * K2 * 3], f32, tag="Dsb")
        d1w_view = d1w_d[:].rearrange("(t n m) p -> n t (m p)", t=T, n=N, m=K2)
        dma(D_sb[:].rearrange("n (t mp) -> n t mp", t=T), d1w_view)

        # ---- small constant loads ----
        ident = cp.tile([128, 128], f32, tag="ident")
        dma(ident[:], ident_d[:])
        scale1 = cp.tile([128, 1], f32, tag="scale1")
        dma(scale1[:], scale1_d[:])
        bias0 = cp.tile([128, 1], f32, tag="bias0")
        dma(bias0[:], bias0_d[:])
        bias1 = cp.tile([128, 1], f32, tag="bias1")
        dma(bias1[:], bias1_d[:])
        wk0 = cp.tile([F, 128], f32, tag="wk0")
        dma(wk0[:], wk0_d[:])
        wk1 = cp.tile([F, 128], f32, tag="wk1")
        dma(wk1[:], wk1_d[:])
        wr0 = cp.tile([U, 128], f32, tag="wr0")
        dma(wr0[:], wr0_d[:])
        wr1 = cp.tile([U, 128], f32, tag="wr1")
        dma(wr1[:], wr1_d[:])
        w1p = cp.tile([U, K1], f32, tag="w1p")
        dma(w1p[:], w1p_d[:])
        c1 = cp.tile([K1, 1], f32, tag="c1")
        dma(c1[:], c1_d[:])
        c1n = cp.tile([K1, 1], f32, tag="c1n")
        dma(c1n[:], c1n_d[:])
        w2rep = cp.tile([K1, 96], f32, tag="w2rep")
        dma(w2rep[:], w2rep_d[:])
        sel96 = cp.tile([96, 3], f32, tag="sel96")
        dma(sel96[:], sel96_d[:])
        ones128 = cp.tile([128, 1], f32, tag="ones128")
        dma(ones128[:], ones128_d[:])
        ones14 = cp.tile([1, BL], f32, tag="ones14")
        dma(ones14[:], ones14_d[:])
        d1b3 = cp.tile([3, 1], f32, tag="d1b3")
        dma(d1b3[:], d1b_d[:])
        d2w = cp.tile([3, N], f32, tag="d2w")
        dma(d2w[:], d2w_d[:])
        d2b = cp.tile([1, N], f32, tag="d2b")
        dma(d2b[:], d2b_d[:])

        # ---- x load + transpose + xz precompute ----
        # x_bt tiles [128 (b,t), 128 (f)] -> PE transpose -> xt [f, bt]
        xt_sb = []
        for i in range(2):
            x_bt = wp.tile([128, F], f32, tag="xbt")
            dma(x_bt[:], x_d[i * 128:(i + 1) * 128])
            tp = pt.tile([128, 128], f32, tag="tp")
            nc.tensor.transpose(tp[:], x_bt[:], ident[:])
            xt = cp.tile([128, 128], f32, tag=f"xt{i}")
            nc.scalar.copy(xt[:], tp[:])
            xt_sb.append(xt)

        # xz[blk] = Wk_blk.T @ xT  scattered to b-major pad layout [128, 260]
        xzt = []
        for blk, wk in ((0, wk0), (1, wk1)):
            xzp = pt.tile([128, CW], f32, tag="tp")
            xzp3 = xzp[:].rearrange("p (b t) -> p b t", b=BL)
            for b in range(BL):
                i, bl = divmod(b, 2)
                nc.tensor.matmul(
                    xzp[:, b * (T + 1) + 1:b * (T + 1) + 1 + T],
                    wk[:], xt_sb[i][:, bl * T:(bl + 1) * T],
                    start=True, stop=True,
                )
            xz_sb = cp.tile([128, CW], f32, tag=f"xzt{blk}")
            xz3 = xz_sb[:].rearrange("p (b t) -> p b t", b=BL)
            nc.scalar.copy(xz3[:, :, 1:T + 1], xzp3[:, :, 1:T + 1])
            nc.vector.memset(xz3[:, :, 0:1], NEG)
            xzt.append(xz_sb)

        # ---- A prep: AT, r = A@1, w = A@r ; wcols [128 (n), 4 (b)] ----
        wcols = cp.tile([128, BL], f32, tag="wcols")
        for b in range(BL):
            a_sb = wp.tile([128, N], f32, tag="asb")
            dma(a_sb[:], a_d[b])
            tp = pt.tile([128, 128], f32, tag="tp")
            nc.tensor.transpose(tp[:], a_sb[:], ident[:])
            at_sb = wp.tile([128, N], f32, tag="atsb")
            nc.scalar.copy(at_sb[:], tp[:])
            rp = ps.tile([128, 1], f32, tag="small")
            nc.tensor.matmul(rp[:], at_sb[:], ones128[:], start=True, stop=True)
            r_sb = wp.tile([128, 1], f32, tag="rsb")
            nc.scalar.copy(r_sb[:], rp[:])
            wpm = ps.tile([128, 1], f32, tag="small")
            nc.tensor.matmul(wpm[:], at_sb[:], r_sb[:], start=True, stop=True)
            nc.scalar.copy(wcols[:, b:b + 1], wpm[:])

        # M1T psum banks: [(m,p)=96, (t-local, b) = 128] x2, filled during LSTM
        m1t = [
            pm.tile([96, 32 * BL], f32, tag="m1t", name=f"m1t{i}")
            for i in range(2)
        ]

        # ---- LSTM Picard iterations ----
        h = wp.tile([U, CW + 1], f32, tag="h")
        nc.vector.memset(h[:], 0.0)

        n_m1_iters = 4  # trace M1T matmuls inside the last 4 iterations
        m1_per_iter = T // n_m1_iters

        for it in range(N_ITERS):
            zp = []
            for blk, wr, xz_sb in ((0, wr0, xzt[0]), (1, wr1, xzt[1])):
                z = pz.tile([128, CW], f32, tag=f"z{blk}")
                nc.tensor.matmul(z[:], ident[:], xz_sb[:], start=True, stop=False)
                nc.tensor.matmul(
                    z[:], wr[:], h[:, 0:CW],
                    start=False, stop=True,
                )
                zp.append(z)
            # s0 = sigmoid(z_if + b_if): Si rows 0:64, Sf rows 64:128
            s0 = wp.tile([128, CW], f32, tag="s0")
            nc.scalar.activation(s0[:], zp[0][:], Act.Sigmoid,
                                 bias=bias0[:], scale=1.0)
            # s1 = sigmoid(scale1*z_go + b_go'): Sg'=sig(2 z_g) rows 0:64,
            # So rows 64:128
            s1 = wp.tile([128, CW], f32, tag="s1")
            nc.scalar.activation(s1[:], zp[1][:], Act.Sigmoid,
                                 bias=bias1[:], scale=scale1[:])
            u1 = wp.tile([U, CW], f32, tag="u1")
            nc.vector.tensor_tensor(u1[:], s0[0:U], s1[0:U], Alu.mult)
            # v = i*g = 2*Si*Sg' - Si, written at base partition 64 so the
            # scan's two inputs (Sf, v) share a base
            v = wp.tile([128, CW], f32, tag="v")
            nc.vector.scalar_tensor_tensor(
                v[U:128], u1[:], 2.0, s0[0:U], Alu.mult, Alu.subtract)
            c = wp.tile([128, CW], f32, tag="c")
            nc.vector.tensor_tensor_scan(
                c[U:128], s0[U:128], v[U:128], 0.0, Alu.mult, Alu.add)
            sc = wp.tile([128, CW], f32, tag="sc")
            nc.scalar.activation(sc[U:128], c[U:128], Act.Sigmoid,
                                 bias=0.0, scale=2.0)
            u2 = wp.tile([128, CW], f32, tag="u2")
            nc.vector.tensor_tensor(u2[U:128], s1[U:128], sc[U:128], Alu.mult)
            h = wp.tile([U, CW + 1], f32, tag="h")
            nc.vector.scalar_tensor_tensor(
                h[:, 1:CW + 1], u2[U:128], 2.0, s1[U:128],
                Alu.mult, Alu.subtract)
            nc.vector.memset(h[:, 0:1], 0.0)

            # interleave M1T matmuls (need D + wcols only) into late iterations
            k = it - (N_ITERS - n_m1_iters)
            if k >= 0:
                for t in range(k * m1_per_iter, (k + 1) * m1_per_iter):
                    bank, tl = t // 32, t % 32
                    nc.tensor.matmul(
                        m1t[bank][:, tl * BL:(tl + 1) * BL],
                        D_sb[:, t * 96:(t + 1) * 96], wcols[:],
                        start=True, stop=True,
                    )

        # ---- GCN tail ----
        s1p = pt.tile([K1, CW], f32, tag="tp")
        nc.tensor.matmul(s1p[:], w1p[:], h[:, 1:CW + 1], start=True, stop=True)
        # leaky(y) = y + (1-slope)*relu(-y), y = s1 + c1
        rn1 = wp.tile([K1, CW], f32, tag="rn1")
        nc.scalar.activation(rn1[:], s1p[:], Act.Relu, bias=c1n[:], scale=-1.0)
        L1a = wp.tile([K1, CW], f32, tag="L1a")
        nc.vector.scalar_tensor_tensor(
            L1a[:], rn1[:], 1.0 - SLOPE, s1p[:], Alu.mult, Alu.add)
        L1 = wp.tile([K1, CW], f32, tag="L1")
        nc.vector.tensor_scalar_add(L1[:], L1a[:], c1[:])
        qp = pt.tile([96, CW], f32, tag="tp")
        nc.tensor.matmul(qp[:], w2rep[:], L1[:], start=True, stop=True)
        rn2 = wp.tile([96, CW], f32, tag="rn2")
        nc.scalar.activation(rn2[:], qp[:], Act.Relu, bias=0.0, scale=-1.0)
        lq = wp.tile([96, CW], f32, tag="lq")
        nc.vector.scalar_tensor_tensor(
            lq[:], rn2[:], 1.0 - SLOPE, qp[:], Alu.mult, Alu.add)

        # prod/reduce: dsum[(m,p), b] = sum_t lq[(m,p), (b,t)] * m1t[(m,p),(t,b)]
        lqv = lq[:].rearrange("p (b t) -> p t b", b=BL)      # [96, 65, 4]
        dparts = []
        for bank in range(2):
            prod = wp.tile([96, 32 * BL], f32, tag="prod")
            pv = prod[:].rearrange("p (t b) -> p t b", b=BL)  # [96, 32, 4]
            nc.vector.tensor_tensor(
                pv[:], lqv[:, 1 + bank * 32:1 + bank * 32 + 32, :],
                m1t[bank][:].rearrange("p (t b) -> p t b", b=BL), Alu.mult)
            dp = wp.tile([96, BL], f32, tag="dpart")
            nc.vector.tensor_reduce(
                dp[:], prod[:].rearrange("p (t b) -> p b t", b=BL),
                mybir.AxisListType.X, Alu.add)
            dparts.append(dp)
        dsum = wp.tile([96, BL], f32, tag="dsum")
        nc.vector.tensor_tensor(dsum[:], dparts[0][:], dparts[1][:], Alu.add)

        d1p = ps.tile([3, BL], f32, tag="small")
        nc.tensor.matmul(d1p[:], sel96[:], dsum[:], start=True, stop=True)
        d1r = wp.tile([3, BL], f32, tag="d1r")
        nc.scalar.activation(d1r[:], d1p[:], Act.Relu, bias=d1b3[:], scale=1.0)

        op = ps.tile([BL, N], f32, tag="small")
        nc.tensor.matmul(op[:], d1r[:], d2w[:], start=True, stop=False)
        nc.tensor.matmul(op[:], ones14[:], d2b[:], start=False, stop=True)
        out_sb = wp.tile([BL, N], f32, tag="outsb")
        nc.scalar.copy(out_sb[:], op[:])
        dma(out_d[:], out_sb[:])

    nc.compile()
    return nc


def fold_inputs(inputs):
    """Host-side weight folding. Returns the per-core-common input dict."""
    f32 = np.float32
    g = {k: np.asarray(v, f32) for k, v in inputs.items()}
    Wk, Wr, lb = g["lstm_k"], g["lstm_r"], g["lstm_b"]

    blk0 = np.arange(2 * U)            # (i, f)
    blk1 = 2 * U + np.arange(2 * U)    # (g, o)
    gsc = np.concatenate([2 * np.ones(U, f32), np.ones(U, f32)])

    sl = g["bnl_g"] / np.sqrt(g["bnl_v"] + EPS)
    tl = g["bnl_b"] - g["bnl_m"] * sl
    g1s = g["bn1_g"] / np.sqrt(g["bn1_v"] + EPS)
    d1s = g["bn1_b"] - g["bn1_m"] * g1s
    g2s = g["bn2_g"] / np.sqrt(g["bn2_v"] + EPS)
    d2s = g["bn2_b"] - g["bn2_m"] * g2s

    # structural requirements of the rank-1 GCN collapse
    assert np.abs(g["b1"]).max() == 0.0, "kernel requires b1 == 0"
    assert np.abs(d1s @ g["w2"]).max() < 1e-30, "kernel requires bn1 shift @ w2 == 0"
    assert np.abs(g["b2"]).max() == 0.0, "kernel requires b2 == 0"
    assert (g2s > 0).all(), "kernel requires positive bn2 scale"

    w2pp = (g1s[:, None] * g["w2"]) * g2s[None, :]
    D4 = g["d1_w"].reshape(T, N, K2, 3)
    constp = np.einsum("m,tnmp->p", d2s, D4)

    return {
        "wk0": np.ascontiguousarray(Wk[:, blk0]),
        "wk1": np.ascontiguousarray(Wk[:, blk1]),
        "wr0": np.ascontiguousarray(Wr[:, blk0]),
        "wr1": np.ascontiguousarray(Wr[:, blk1]),
        "bias0": np.ascontiguousarray(lb[blk0].reshape(128, 1)),
        "bias1": np.ascontiguousarray((gsc * lb[blk1]).reshape(128, 1)),
        "w1p": np.ascontiguousarray(sl[:, None] * g["w1"]),
        "c1": np.ascontiguousarray((tl @ g["w1"]).reshape(K1, 1)),
        "c1n": np.ascontiguousarray((-(tl @ g["w1"])).reshape(K1, 1)),
        "w2rep": np.ascontiguousarray(np.repeat(w2pp, 3, axis=1)),
        "d1w": np.ascontiguousarray(g["d1_w"]),
        "d1b3": np.ascontiguousarray((g["d1_b"] + constp).reshape(3, 1)),
        "d2w": np.ascontiguousarray(g["d2_w"]),
        "d2b": np.ascontiguousarray(g["d2_b"].reshape(1, N)),
    }


def make_in_maps(inputs):
    common = fold_inputs(inputs)
    x = np.asarray(inputs["x"], np.float32)
    a = np.asarray(inputs["a"], np.float32)
    in_maps = []
    for core in range(NCORE):
        m = dict(common)
        m["x_sh"] = np.ascontiguousarray(
            x[core * BL:(core + 1) * BL].reshape(BL * T, F))
        m["a_sh"] = np.ascontiguousarray(a[core * BL:(core + 1) * BL])
        in_maps.append(m)
    return in_maps


def kernel(**inputs):
    from concourse.bass_utils import run_bass_kernel_spmd

    if "module" not in _CACHE:
        _CACHE["module"] = build_module()
    nc = _CACHE["module"]

    in_maps = make_in_maps(inputs)
    res = run_bass_kernel_spmd(nc, in_maps, core_ids=list(range(NCORE)))
    out = np.concatenate([res.results[i]["out_sh"] for i in range(NCORE)], axis=0)
    return out.astype(np.float32)


# revision 11
# speedup vs baseline: 1.2588x; 1.2588x over previous
"""Trainium2 Bass kernel for nn_EndToEndCryptoModel (LSTM -> GCNx2 -> Dense).

Strategy (per-core, data-parallel over batch, 4 batches/core on 8 cores):
  * LSTM solved by Picard fixed-point iteration over the whole sequence:
    7 iterations, each fully parallel over (b, t) using big sigmoid ops
    (all 4 gates via sigmoid; tanh(y) = 2*sigmoid(2y)-1 with the 2x folded
    into weights / per-partition ACT scale), and the cell-state recurrence
    done by a single DVE tensor_tensor_scan along the time axis (batch
    chains separated by poison pad columns that reset the scan).
  * The GCN collapses algebraically: sup1 is node-independent, so
    g1 = leaky(rowsum(a) (x) s1) is rank-1 (leaky is positively homogeneous
    and b1 == 0), and the whole two-layer GCN reduces to per-(t,m) scalars
    q[t,m] and per-node weights w[n] = (a @ (a @ 1))[n].
  * Final dense layer: d1[b,p] = sum_{t,m} Lq'[b,t,m] * (w[b,:] @ D[t,:,m,p]),
    computed as 64 small matmuls with D t-slices as stationary weights into a
    [(m,p)=96, (t,b)] PSUM layout that exactly matches the layout q comes in
    (w2 columns pre-replicated x3 on the host), then DVE multiply+reduce and
    two tiny matmuls.

All heavy layout decisions are hardcoded for the fixed problem shapes.
"""

import numpy as np

B, T, N, F = 32, 64, 128, 128
U, K1, K2 = 64, 64, 32
NCORE = 8
BL = B // NCORE            # 4 batches per core
CW = BL * (T + 1)          # 260 columns, b-major with pad col at b*(T+1)
NEG = -1e30
EPS = 1e-3
SLOPE = 0.01
N_ITERS = 7

_CACHE = {}

# constant-bundle column layout: name -> (col_off, rows, cols)
_BUNDLE = {}
_off = 0
for _name, _rows, _cols in [
    ("ident", 128, 128), ("wk0", 128, 128), ("wk1", 128, 128),
    ("wr0", 64, 128), ("wr1", 64, 128), ("w1p", 64, 64), ("w2rep", 64, 96),
    ("d2w", 3, 128), ("sel96", 96, 3), ("d2b", 1, 128), ("ones14", 1, 4),
    ("bias0", 128, 1), ("bias1", 128, 1), ("scale1", 128, 1),
    ("c1", 64, 1), ("c1n", 64, 1), ("ones128", 128, 1), ("d1b3", 3, 1),
]:
    _BUNDLE[_name] = (_off, _rows, _cols)
    _off += _cols
BUNDLE_W = _off


def build_module(fast_z=False, fast_m1=False):
    """Build the per-core Bass/Tile module (identical SPMD program).

    fast_z: use float32r for the per-iteration z matmuls (xz + Wr@h).
    fast_m1: use float32r for the M1T (dense-layer) matmuls.
    """
    from contextlib import ExitStack
    import concourse.bacc as bacc
    import concourse.mybir as mybir
    from concourse import tile

    f32 = mybir.dt.float32
    f32r = mybir.dt.float32r
    zdt = f32r if fast_z else f32
    mdt = f32r if fast_m1 else f32
    Alu = mybir.AluOpType
    Act = mybir.ActivationFunctionType

    nc = bacc.Bacc(None, target_bir_lowering=False)

    # ---------------- DRAM I/O ----------------
    x_d = nc.dram_tensor("x_sh", [BL * T, F], f32, kind="ExternalInput")
    a_d = nc.dram_tensor("a_sh", [BL, N, N], f32, kind="ExternalInput")
    cb_d = nc.dram_tensor("cbundle", [128, BUNDLE_W], f32, kind="ExternalInput")
    d1w_d = nc.dram_tensor("d1w", [T * N * K2, 3], f32, kind="ExternalInput")
    out_d = nc.dram_tensor("out_sh", [BL, N], f32, kind="ExternalOutput")

    with tile.TileContext(nc) as tc, ExitStack() as ctx:
        cp = ctx.enter_context(tc.tile_pool(name="const", bufs=1))
        wp = ctx.enter_context(tc.tile_pool(name="work", bufs=2))
        pz = ctx.enter_context(tc.tile_pool(name="pz", bufs=1, space="PSUM"))
        pm = ctx.enter_context(tc.tile_pool(name="pm", bufs=2, space="PSUM"))
        pt = ctx.enter_context(tc.tile_pool(name="pt", bufs=2, space="PSUM"))
        ps = ctx.enter_context(tc.tile_pool(name="ps", bufs=2, space="PSUM"))

        dma = nc.sync.dma_start

        # ---- DMAs: constants bundle + x first, then a; big D last ----
        cb = cp.tile([128, BUNDLE_W], f32, tag="cb")
        dma(cb[:], cb_d[:])

        def cview(name):
            off, rows, cols = _BUNDLE[name]
            return cb[0:rows, off:off + cols]

        x2 = wp.tile([128, 256], f32, tag="x2")
        dma(x2[:].rearrange("p (i f) -> p i f", i=2),
            x_d[:].rearrange("(i p) f -> p i f", i=2))

        a_all = wp.tile([128, BL * N], f32, tag="a_all")
        dma(a_all[:].rearrange("p (b n) -> p b n", b=BL),
            a_d[:].rearrange("b p n -> p b n"))

        D_sb = cp.tile([128, T * K2 * 3], mdt, tag="Dsb")
        d1w_view = d1w_d[:].rearrange("(t n m) p -> n t (m p)", t=T, n=N, m=K2)
        dma(D_sb[:].rearrange("n (t mp) -> n t mp", t=T), d1w_view)

        ident = cview("ident")
        if fast_z:
            identr = cp.tile([128, 128], f32r, tag="identr")
            nc.vector.tensor_copy(identr[:], ident)
            ident_z = identr[:]
        else:
            ident_z = ident

        # ---- x transpose + xz precompute ----
        xt_sb = []
        for i in range(2):
            tp = pt.tile([128, 128], f32, tag="tp")
            nc.tensor.transpose(tp[:], x2[:, i * 128:(i + 1) * 128], ident)
            xt = cp.tile([128, 128], f32, tag=f"xt{i}")
            nc.vector.tensor_copy(xt[:], tp[:])
            xt_sb.append(xt)

        # xz[blk] = Wk_blk.T @ xT  scattered to b-major pad layout [128, 260]
        xzt = []
        for blk in range(2):
            wk = cview("wk0" if blk == 0 else "wk1")
            xzp = pt.tile([128, CW], f32, tag="tp")
            xzp3 = xzp[:].rearrange("p (b t) -> p b t", b=BL)
            for b in range(BL):
                i, bl = divmod(b, 2)
                nc.tensor.matmul(
                    xzp[:, b * (T + 1) + 1:b * (T + 1) + 1 + T],
                    wk[:], xt_sb[i][:, bl * T:(bl + 1) * T],
                    start=True, stop=True,
                )
            xz_sb = cp.tile([128, CW], zdt, tag=f"xzt{blk}")
            xz3 = xz_sb[:].rearrange("p (b t) -> p b t", b=BL)
            nc.vector.tensor_copy(xz3[:, :, 1:T + 1], xzp3[:, :, 1:T + 1])
            nc.vector.memset(xz3[:, :, 0:1], NEG)
            xzt.append(xz_sb)

        # ---- A prep: AT, r = A@1, w = A@r ; wcols [128 (n), 4 (b)] ----
        wcols = cp.tile([128, BL], mdt, tag="wcols")
        ones128 = cview("ones128")
        for b in range(BL):
            tp = pt.tile([128, 128], f32, tag="tp")
            nc.tensor.transpose(tp[:], a_all[:, b * N:(b + 1) * N], ident)
            at_sb = wp.tile([128, N], f32, tag="atsb")
            nc.vector.tensor_copy(at_sb[:], tp[:])
            rp = ps.tile([128, 1], f32, tag="small")
            nc.tensor.matmul(rp[:], at_sb[:], ones128, start=True, stop=True)
            r_sb = wp.tile([128, 1], f32, tag="rsb")
            nc.vector.tensor_copy(r_sb[:], rp[:])
            wpm = ps.tile([128, 1], f32, tag="small")
            nc.tensor.matmul(wpm[:], at_sb[:], r_sb[:], start=True, stop=True)
            nc.vector.tensor_copy(wcols[:, b:b + 1], wpm[:])

        # M1T psum banks: [(m,p)=96, (t-local, b) = 128] x2, filled during LSTM
        m1t = [
            pm.tile([96, 32 * BL], f32, tag="m1t", name=f"m1t{i}")
            for i in range(2)
        ]

        # ---- LSTM Picard iterations ----
        bias0, bias1, scale1 = cview("bias0"), cview("bias1"), cview("scale1")
        wr0z = cp.tile([U, 128], zdt, tag="wr0z")
        nc.vector.tensor_copy(wr0z[:], cview("wr0"))
        wr1z = cp.tile([U, 128], zdt, tag="wr1z")
        nc.vector.tensor_copy(wr1z[:], cview("wr1"))

        h = None
        m1_sched = {2: 13, 3: 13, 4: 13, 5: 13, 6: 12}
        m1_done = 0

        for it in range(N_ITERS):
            zp = []
            for blk, wr, xz_sb in ((0, wr0z, xzt[0]), (1, wr1z, xzt[1])):
                z = pz.tile([128, CW], f32, tag=f"z{blk}")
                if it == 0:
                    nc.tensor.matmul(z[:], ident_z, xz_sb[:],
                                     start=True, stop=True)
                else:
                    nc.tensor.matmul(z[:], ident_z, xz_sb[:],
                                     start=True, stop=False)
                    nc.tensor.matmul(z[:], wr[:], h[:, 0:CW],
                                     start=False, stop=True)
                zp.append(z)
            # s0 = sigmoid(z_if + b_if): Si rows 0:64, Sf rows 64:128
            s0 = wp.tile([128, CW], f32, tag="s0")
            nc.scalar.activation(s0[:], zp[0][:], Act.Sigmoid,
                                 bias=bias0, scale=1.0)
            # s1 = sigmoid(scale1*z_go + b_go'): Sg'=sig(2 z_g) rows 0:64,
            # So rows 64:128
            s1 = wp.tile([128, CW], f32, tag="s1")
            nc.scalar.activation(s1[:], zp[1][:], Act.Sigmoid,
                                 bias=bias1, scale=scale1)
            u1 = wp.tile([U, CW], f32, tag="u1")
            nc.vector.tensor_tensor(u1[:], s0[0:U], s1[0:U], Alu.mult)
            # v = i*g = 2*Si*Sg' - Si, written at base partition 64 so the
            # scan's two inputs (Sf, v) share a base
            v = wp.tile([128, CW], f32, tag="v")
            nc.vector.scalar_tensor_tensor(
                v[U:128], u1[:], 2.0, s0[0:U], Alu.mult, Alu.subtract)
            c = wp.tile([128, CW], f32, tag="c")
            nc.vector.tensor_tensor_scan(
                c[U:128], s0[U:128], v[U:128], 0.0, Alu.mult, Alu.add)
            sc = wp.tile([128, CW], f32, tag="sc")
            nc.scalar.activation(sc[U:128], c[U:128], Act.Sigmoid,
                                 bias=0.0, scale=2.0)
            u2 = wp.tile([128, CW], f32, tag="u2")
            nc.vector.tensor_tensor(u2[U:128], s1[U:128], sc[U:128], Alu.mult)
            h = wp.tile([U, CW + 1], zdt, tag="h")
            nc.vector.scalar_tensor_tensor(
                h[:, 1:CW + 1], u2[U:128], 2.0, s1[U:128],
                Alu.mult, Alu.subtract)
            nc.vector.memset(h[:, 0:1], 0.0)

            # interleave M1T matmuls (need D + wcols only) into mid iterations
            for t in range(m1_done, m1_done + m1_sched.get(it, 0)):
                bank, tl = t // 32, t % 32
                nc.tensor.matmul(
                    m1t[bank][:, tl * BL:(tl + 1) * BL],
                    D_sb[:, t * 96:(t + 1) * 96], wcols[:],
                    start=True, stop=True,
                )
            m1_done += m1_sched.get(it, 0)

        # ---- GCN tail ----
        if fast_z:
            w1pz = cp.tile([U, K1], zdt, tag="w1pz")
            nc.vector.tensor_copy(w1pz[:], cview("w1p"))
            w1p_use = w1pz[:]
        else:
            w1p_use = cview("w1p")
        s1p = pt.tile([K1, CW], f32, tag="tp")
        nc.tensor.matmul(s1p[:], w1p_use, h[:, 1:CW + 1],
                         start=True, stop=True)
        # leaky(y) = y + (1-slope)*relu(-y), y = s1 + c1
        rn1 = wp.tile([K1, CW], f32, tag="rn1")
        nc.scalar.activation(rn1[:], s1p[:], Act.Relu,
                             bias=cview("c1n"), scale=-1.0)
        L1a = wp.tile([K1, CW], f32, tag="L1a")
        nc.vector.scalar_tensor_tensor(
            L1a[:], rn1[:], 1.0 - SLOPE, s1p[:], Alu.mult, Alu.add)
        L1 = wp.tile([K1, CW], f32, tag="L1")
        nc.vector.tensor_scalar_add(L1[:], L1a[:], cview("c1"))
        qp = pt.tile([96, CW], f32, tag="tp")
        nc.tensor.matmul(qp[:], cview("w2rep"), L1[:], start=True, stop=True)
        rn2 = wp.tile([96, CW], f32, tag="rn2")
        nc.scalar.activation(rn2[:], qp[:], Act.Relu, bias=0.0, scale=-1.0)
        lq = wp.tile([96, CW], f32, tag="lq")
        nc.vector.scalar_tensor_tensor(
            lq[:], rn2[:], 1.0 - SLOPE, qp[:], Alu.mult, Alu.add)

        # prod/reduce: dsum[(m,p), b] = sum_t lq[(m,p), (b,t)] * m1t[(m,p),(t,b)]
        lqv = lq[:].rearrange("p (b t) -> p t b", b=BL)      # [96, 65, 4]
        dparts = []
        for bank in range(2):
            prod = wp.tile([96, 32 * BL], f32, tag="prod")
            pv = prod[:].rearrange("p (t b) -> p t b", b=BL)  # [96, 32, 4]
            nc.vector.tensor_tensor(
                pv[:], lqv[:, 1 + bank * 32:1 + bank * 32 + 32, :],
                m1t[bank][:].rearrange("p (t b) -> p t b", b=BL), Alu.mult)
            dp = wp.tile([96, BL], f32, tag="dpart")
            nc.vector.tensor_reduce(
                dp[:], prod[:].rearrange("p (t b) -> p b t", b=BL),
                mybir.AxisListType.X, Alu.add)
            dparts.append(dp)
        dsum = wp.tile([96, BL], f32, tag="dsum")
        nc.vector.tensor_tensor(dsum[:], dparts[0][:], dparts[1][:], Alu.add)

        d1p = ps.tile([3, BL], f32, tag="small")
        nc.tensor.matmul(d1p[:], cview("sel96"), dsum[:], start=True, stop=True)
        d1r = wp.tile([3, BL], f32, tag="d1r")
        nc.scalar.activation(d1r[:], d1p[:], Act.Relu,
                             bias=cview("d1b3"), scale=1.0)

        op = ps.tile([BL, N], f32, tag="small")
        nc.tensor.matmul(op[:], d1r[:], cview("d2w"), start=True, stop=False)
        nc.tensor.matmul(op[:], cview("ones14"), cview("d2b"),
                         start=False, stop=True)
        out_sb = wp.tile([BL, N], f32, tag="outsb")
        nc.scalar.copy(out_sb[:], op[:])
        dma(out_d[:], out_sb[:])

    nc.compile()
    return nc


def fold_inputs(inputs):
    """Host-side weight folding. Returns the per-core-common input dict."""
    f32 = np.float32
    g = {k: np.asarray(v, f32) for k, v in inputs.items()}
    Wk, Wr, lb = g["lstm_k"], g["lstm_r"], g["lstm_b"]

    blk0 = np.arange(2 * U)            # (i, f)
    blk1 = 2 * U + np.arange(2 * U)    # (g, o)
    gsc = np.concatenate([2 * np.ones(U, f32), np.ones(U, f32)])

    sl = g["bnl_g"] / np.sqrt(g["bnl_v"] + EPS)
    tl = g["bnl_b"] - g["bnl_m"] * sl
    g1s = g["bn1_g"] / np.sqrt(g["bn1_v"] + EPS)
    d1s = g["bn1_b"] - g["bn1_m"] * g1s
    g2s = g["bn2_g"] / np.sqrt(g["bn2_v"] + EPS)
    d2s = g["bn2_b"] - g["bn2_m"] * g2s

    # structural requirements of the rank-1 GCN collapse
    assert np.abs(g["b1"]).max() == 0.0, "kernel requires b1 == 0"
    assert np.abs(d1s @ g["w2"]).max() < 1e-30, "kernel requires bn1 shift @ w2 == 0"
    assert np.abs(g["b2"]).max() == 0.0, "kernel requires b2 == 0"
    assert (g2s > 0).all(), "kernel requires positive bn2 scale"

    w2pp = (g1s[:, None] * g["w2"]) * g2s[None, :]
    D4 = g["d1_w"].reshape(T, N, K2, 3)
    constp = np.einsum("m,tnmp->p", d2s, D4)
    c1v = tl @ g["w1"]

    vals = {
        "ident": np.eye(128, dtype=f32),
        "wk0": Wk[:, blk0], "wk1": Wk[:, blk1],
        "wr0": Wr[:, blk0], "wr1": Wr[:, blk1],
        "w1p": sl[:, None] * g["w1"],
        "w2rep": np.repeat(w2pp, 3, axis=1),
        "d2w": g["d2_w"],
        "sel96": np.kron(np.ones((K2, 1), f32), np.eye(3, dtype=f32)),
        "d2b": g["d2_b"].reshape(1, N),
        "ones14": np.ones((1, BL), f32),
        "bias0": lb[blk0].reshape(128, 1),
        "bias1": (gsc * lb[blk1]).reshape(128, 1),
        "scale1": np.concatenate(
            [2 * np.ones(U, f32), np.ones(U, f32)]).reshape(128, 1),
        "c1": c1v.reshape(K1, 1),
        "c1n": (-c1v).reshape(K1, 1),
        "ones128": np.ones((128, 1), f32),
        "d1b3": (g["d1_b"] + constp).reshape(3, 1),
    }
    cb = np.zeros((128, BUNDLE_W), f32)
    for name, (off, rows, cols) in _BUNDLE.items():
        v = vals[name]
        assert v.shape == (rows, cols), (name, v.shape, (rows, cols))
        cb[0:rows, off:off + cols] = v
    return {"cbundle": cb, "d1w": np.ascontiguousarray(g["d1_w"])}


def make_in_maps(inputs):
    common = fold_inputs(inputs)
    x = np.asarray(inputs["x"], np.float32)
    a = np.asarray(inputs["a"], np.float32)
    in_maps = []
    for core in range(NCORE):
        m = dict(common)
        m["x_sh"] = np.ascontiguousarray(
            x[core * BL:(core + 1) * BL].reshape(BL * T, F))
        m["a_sh"] = np.ascontiguousarray(a[core * BL:(core + 1) * BL])
        in_maps.append(m)
    return in_maps


def kernel(**inputs):
    from concourse.bass_utils import run_bass_kernel_spmd

    if "module" not in _CACHE:
        _CACHE["module"] = build_module()
    nc = _CACHE["module"]

    in_maps = make_in_maps(inputs)
    res = run_bass_kernel_spmd(nc, in_maps, core_ids=list(range(NCORE)))
    out = np.concatenate([res.results[i]["out_sh"] for i in range(NCORE)], axis=0)
    return out.astype(np.float32)


# revision 13
# speedup vs baseline: 1.5738x; 1.2502x over previous
"""Trainium2 Bass kernel for nn_EndToEndCryptoModel (LSTM -> GCNx2 -> Dense).

Strategy (per-core, data-parallel over batch, 4 batches/core on 8 cores):
  * LSTM solved by Picard fixed-point iteration over the whole sequence:
    7 iterations, each fully parallel over (b, t) using big sigmoid ops
    (all 4 gates via sigmoid; tanh(y) = 2*sigmoid(2y)-1 with the 2x folded
    into weights / per-partition ACT scale), and the cell-state recurrence
    done by a single DVE tensor_tensor_scan along the time axis (batch
    chains separated by poison pad columns that reset the scan).
  * The GCN collapses algebraically: sup1 is node-independent, so
    g1 = leaky(rowsum(a) (x) s1) is rank-1 (leaky is positively homogeneous
    and b1 == 0), and the whole two-layer GCN reduces to per-(t,m) scalars
    q[t,m] and per-node weights w[n] = (a @ (a @ 1))[n].
  * Final dense layer: d1[b,p] = sum_{t,m} Lq'[b,t,m] * (w[b,:] @ D[t,:,m,p]),
    computed as 64 small matmuls with D t-slices as stationary weights into a
    [(m,p)=96, (t,b)] PSUM layout that exactly matches the layout q comes in
    (w2 columns pre-replicated x3 on the host), then DVE multiply+reduce and
    two tiny matmuls.

All heavy layout decisions are hardcoded for the fixed problem shapes.
"""

import numpy as np

B, T, N, F = 32, 64, 128, 128
U, K1, K2 = 64, 64, 32
NCORE = 8
BL = B // NCORE            # 4 batches per core
CW = BL * (T + 1)          # 260 columns, b-major with pad col at b*(T+1)
NEG = -1e30
EPS = 1e-3
SLOPE = 0.01
N_ITERS = 7

_CACHE = {}

# constant-bundle column layout: name -> (col_off, rows, cols)
_BUNDLE = {}
_off = 0
for _name, _rows, _cols in [
    ("ident", 128, 128), ("wk0", 128, 128), ("wk1", 128, 128),
    ("wr0", 64, 128), ("wr1", 64, 128), ("w1p", 64, 64), ("w2rep", 64, 96),
    ("d2w", 3, 128), ("sel96", 96, 3), ("d2b", 1, 128), ("ones14", 1, 4),
    ("bias0", 128, 1), ("bias1", 128, 1), ("scale1", 128, 1),
    ("c1", 64, 1), ("c1n", 64, 1), ("ones128", 128, 1), ("d1b3", 3, 1),
]:
    _BUNDLE[_name] = (_off, _rows, _cols)
    _off += _cols
BUNDLE_W = _off


def build_module(fast_z=False, fast_m1=False):
    """Build the per-core Bass/Tile module (identical SPMD program).

    fast_z: use float32r for the per-iteration z matmuls (xz + Wr@h).
    fast_m1: use float32r for the M1T (dense-layer) matmuls.
    """
    from contextlib import ExitStack
    import concourse.bacc as bacc
    import concourse.mybir as mybir
    from concourse import tile

    f32 = mybir.dt.float32
    f32r = mybir.dt.float32r
    zdt = f32r if fast_z else f32
    mdt = f32r if fast_m1 else f32
    Alu = mybir.AluOpType
    Act = mybir.ActivationFunctionType

    nc = bacc.Bacc(None, target_bir_lowering=False)

    # ---------------- DRAM I/O ----------------
    x_d = nc.dram_tensor("x_sh", [BL * T, F], f32, kind="ExternalInput")
    a_d = nc.dram_tensor("a_sh", [BL, N, N], f32, kind="ExternalInput")
    cb_d = nc.dram_tensor("cbundle", [128, BUNDLE_W], f32, kind="ExternalInput")
    d1w_d = nc.dram_tensor("d1w", [T * N * K2, 3], mdt, kind="ExternalInput")
    out_d = nc.dram_tensor("out_sh", [BL, N], f32, kind="ExternalOutput")

    with tile.TileContext(nc) as tc, ExitStack() as ctx:
        cp = ctx.enter_context(tc.tile_pool(name="const", bufs=1))
        wp = ctx.enter_context(tc.tile_pool(name="work", bufs=2))
        pz = ctx.enter_context(tc.tile_pool(name="pz", bufs=1, space="PSUM"))
        pm = ctx.enter_context(tc.tile_pool(name="pm", bufs=2, space="PSUM"))
        pt = ctx.enter_context(tc.tile_pool(name="pt", bufs=2, space="PSUM"))
        ps = ctx.enter_context(tc.tile_pool(name="ps", bufs=2, space="PSUM"))

        dma = nc.sync.dma_start

        # ---- DMAs: constants bundle + x first, then a; big D last ----
        cb = cp.tile([128, BUNDLE_W], f32, tag="cb")
        dma(cb[:], cb_d[:])

        def cview(name):
            off, rows, cols = _BUNDLE[name]
            return cb[0:rows, off:off + cols]

        x2 = wp.tile([128, 256], f32, tag="x2")
        dma(x2[:].rearrange("p (i f) -> p i f", i=2),
            x_d[:].rearrange("(i p) f -> p i f", i=2))

        a_all = wp.tile([128, BL * N], f32, tag="a_all")
        dma(a_all[:].rearrange("p (b n) -> p b n", b=BL),
            a_d[:].rearrange("b p n -> p b n"))

        D_sb = cp.tile([128, T * K2 * 3], mdt, tag="Dsb")
        d1w_view = d1w_d[:].rearrange("(t n m) p -> n t (m p)", t=T, n=N, m=K2)
        dma(D_sb[:].rearrange("n (t mp) -> n t mp", t=T), d1w_view)

        ident = cview("ident")
        if fast_z:
            identr = cp.tile([128, 128], f32r, tag="identr")
            nc.vector.tensor_copy(identr[:], ident)
            ident_z = identr[:]
        else:
            ident_z = ident

        # ---- x transpose + xz precompute ----
        xt_sb = []
        for i in range(2):
            tp = pt.tile([128, 128], f32, tag="tp")
            nc.tensor.transpose(tp[:], x2[:, i * 128:(i + 1) * 128], ident)
            xt = cp.tile([128, 128], f32, tag=f"xt{i}")
            nc.vector.tensor_copy(xt[:], tp[:])
            xt_sb.append(xt)

        # xz[blk] = Wk_blk.T @ xT  scattered to b-major pad layout [128, 260]
        xzt = []
        for blk in range(2):
            wk = cview("wk0" if blk == 0 else "wk1")
            xzp = pt.tile([128, CW], f32, tag="tp")
            xzp3 = xzp[:].rearrange("p (b t) -> p b t", b=BL)
            for b in range(BL):
                i, bl = divmod(b, 2)
                nc.tensor.matmul(
                    xzp[:, b * (T + 1) + 1:b * (T + 1) + 1 + T],
                    wk[:], xt_sb[i][:, bl * T:(bl + 1) * T],
                    start=True, stop=True,
                )
            xz_sb = cp.tile([128, CW], zdt, tag=f"xzt{blk}")
            xz3 = xz_sb[:].rearrange("p (b t) -> p b t", b=BL)
            nc.vector.tensor_copy(xz3[:, :, 1:T + 1], xzp3[:, :, 1:T + 1])
            nc.vector.memset(xz3[:, :, 0:1].bitcast(f32), NEG)
            xzt.append(xz_sb)

        # ---- A prep: AT, r = A@1, w = A@r ; wcols [128 (n), 4 (b)] ----
        wcols = cp.tile([128, BL], mdt, tag="wcols")
        ones128 = cview("ones128")
        for b in range(BL):
            tp = pt.tile([128, 128], f32, tag="tp")
            nc.tensor.transpose(tp[:], a_all[:, b * N:(b + 1) * N], ident)
            at_sb = wp.tile([128, N], f32, tag="atsb")
            nc.vector.tensor_copy(at_sb[:], tp[:])
            rp = ps.tile([128, 1], f32, tag="small")
            nc.tensor.matmul(rp[:], at_sb[:], ones128, start=True, stop=True)
            r_sb = wp.tile([128, 1], f32, tag="rsb")
            nc.vector.tensor_copy(r_sb[:], rp[:])
            wpm = ps.tile([128, 1], f32, tag="small")
            nc.tensor.matmul(wpm[:], at_sb[:], r_sb[:], start=True, stop=True)
            nc.vector.tensor_copy(wcols[:, b:b + 1], wpm[:])

        # M1T psum banks: [(m,p)=96, (t-local, b) = 128] x2, filled during LSTM
        m1t = [
            pm.tile([96, 32 * BL], f32, tag="m1t", name=f"m1t{i}")
            for i in range(2)
        ]

        # ---- LSTM Picard iterations ----
        bias0, bias1, scale1 = cview("bias0"), cview("bias1"), cview("scale1")
        wr0z = cp.tile([U, 128], zdt, tag="wr0z")
        nc.vector.tensor_copy(wr0z[:], cview("wr0"))
        wr1z = cp.tile([U, 128], zdt, tag="wr1z")
        nc.vector.tensor_copy(wr1z[:], cview("wr1"))

        h = None
        m1_sched = {2: 13, 3: 13, 4: 13, 5: 13, 6: 12}
        m1_done = 0

        for it in range(N_ITERS):
            zp = []
            for blk, wr, xz_sb in ((0, wr0z, xzt[0]), (1, wr1z, xzt[1])):
                z = pz.tile([128, CW], f32, tag=f"z{blk}")
                if it == 0:
                    nc.tensor.matmul(z[:], ident_z, xz_sb[:],
                                     start=True, stop=True)
                else:
                    nc.tensor.matmul(z[:], ident_z, xz_sb[:],
                                     start=True, stop=False)
                    nc.tensor.matmul(z[:], wr[:], h[:, 0:CW],
                                     start=False, stop=True)
                zp.append(z)
            # s0 = sigmoid(z_if + b_if): Si rows 0:64, Sf rows 64:128
            s0 = wp.tile([128, CW], f32, tag="s0")
            nc.scalar.activation(s0[:], zp[0][:], Act.Sigmoid,
                                 bias=bias0, scale=1.0)
            # s1 = sigmoid(scale1*z_go + b_go'): Sg'=sig(2 z_g) rows 0:64,
            # So rows 64:128
            s1 = wp.tile([128, CW], f32, tag="s1")
            nc.scalar.activation(s1[:], zp[1][:], Act.Sigmoid,
                                 bias=bias1, scale=scale1)
            u1 = wp.tile([U, CW], f32, tag="u1")
            nc.vector.tensor_tensor(u1[:], s0[0:U], s1[0:U], Alu.mult)
            # v = i*g = 2*Si*Sg' - Si, written at base partition 64 so the
            # scan's two inputs (Sf, v) share a base
            v = wp.tile([128, CW], f32, tag="v")
            nc.vector.scalar_tensor_tensor(
                v[U:128], u1[:], 2.0, s0[0:U], Alu.mult, Alu.subtract)
            c = wp.tile([128, CW], f32, tag="c")
            nc.vector.tensor_tensor_scan(
                c[U:128], s0[U:128], v[U:128], 0.0, Alu.mult, Alu.add)
            sc = wp.tile([128, CW], f32, tag="sc")
            nc.scalar.activation(sc[U:128], c[U:128], Act.Sigmoid,
                                 bias=0.0, scale=2.0)
            u2 = wp.tile([128, CW], f32, tag="u2")
            nc.vector.tensor_tensor(u2[U:128], s1[U:128], sc[U:128], Alu.mult)
            h = wp.tile([U, CW + 1], zdt, tag="h")
            nc.vector.scalar_tensor_tensor(
                h[:, 1:CW + 1], u2[U:128], 2.0, s1[U:128],
                Alu.mult, Alu.subtract)
            nc.vector.memset(h[:, 0:1].bitcast(f32), 0.0)

            # interleave M1T matmuls (need D + wcols only) into mid iterations
            for t in range(m1_done, m1_done + m1_sched.get(it, 0)):
                bank, tl = t // 32, t % 32
                nc.tensor.matmul(
                    m1t[bank][:, tl * BL:(tl + 1) * BL],
                    D_sb[:, t * 96:(t + 1) * 96], wcols[:],
                    start=True, stop=True,
                )
            m1_done += m1_sched.get(it, 0)

        # ---- GCN tail ----
        if fast_z:
            w1pz = cp.tile([U, K1], zdt, tag="w1pz")
            nc.vector.tensor_copy(w1pz[:], cview("w1p"))
            w1p_use = w1pz[:]
        else:
            w1p_use = cview("w1p")
        s1p = pt.tile([K1, CW], f32, tag="tp")
        nc.tensor.matmul(s1p[:], w1p_use, h[:, 1:CW + 1],
                         start=True, stop=True)
        # leaky(y) = y + (1-slope)*relu(-y), y = s1 + c1
        rn1 = wp.tile([K1, CW], f32, tag="rn1")
        nc.scalar.activation(rn1[:], s1p[:], Act.Relu,
                             bias=cview("c1n"), scale=-1.0)
        L1a = wp.tile([K1, CW], f32, tag="L1a")
        nc.vector.scalar_tensor_tensor(
            L1a[:], rn1[:], 1.0 - SLOPE, s1p[:], Alu.mult, Alu.add)
        L1 = wp.tile([K1, CW], f32, tag="L1")
        nc.vector.tensor_scalar_add(L1[:], L1a[:], cview("c1"))
        qp = pt.tile([96, CW], f32, tag="tp")
        nc.tensor.matmul(qp[:], cview("w2rep"), L1[:], start=True, stop=True)
        rn2 = wp.tile([96, CW], f32, tag="rn2")
        nc.scalar.activation(rn2[:], qp[:], Act.Relu, bias=0.0, scale=-1.0)
        lq = wp.tile([96, CW], f32, tag="lq")
        nc.vector.scalar_tensor_tensor(
            lq[:], rn2[:], 1.0 - SLOPE, qp[:], Alu.mult, Alu.add)

        # prod/reduce: dsum[(m,p), b] = sum_t lq[(m,p), (b,t)] * m1t[(m,p),(t,b)]
        lqv = lq[:].rearrange("p (b t) -> p t b", b=BL)      # [96, 65, 4]
        dparts = []
        for bank in range(2):
            prod = wp.tile([96, 32 * BL], f32, tag="prod")
            pv = prod[:].rearrange("p (t b) -> p t b", b=BL)  # [96, 32, 4]
            nc.vector.tensor_tensor(
                pv[:], lqv[:, 1 + bank * 32:1 + bank * 32 + 32, :],
                m1t[bank][:].rearrange("p (t b) -> p t b", b=BL), Alu.mult)
            dp = wp.tile([96, BL], f32, tag="dpart")
            nc.vector.tensor_reduce(
                dp[:], prod[:].rearrange("p (t b) -> p b t", b=BL),
                mybir.AxisListType.X, Alu.add)
            dparts.append(dp)
        dsum = wp.tile([96, BL], f32, tag="dsum")
        nc.vector.tensor_tensor(dsum[:], dparts[0][:], dparts[1][:], Alu.add)

        d1p = ps.tile([3, BL], f32, tag="small")
        nc.tensor.matmul(d1p[:], cview("sel96"), dsum[:], start=True, stop=True)
        d1r = wp.tile([3, BL], f32, tag="d1r")
        nc.scalar.activation(d1r[:], d1p[:], Act.Relu,
                             bias=cview("d1b3"), scale=1.0)

        op = ps.tile([BL, N], f32, tag="small")
        nc.tensor.matmul(op[:], d1r[:], cview("d2w"), start=True, stop=False)
        nc.tensor.matmul(op[:], cview("ones14"), cview("d2b"),
                         start=False, stop=True)
        out_sb = wp.tile([BL, N], f32, tag="outsb")
        nc.scalar.copy(out_sb[:], op[:])
        dma(out_d[:], out_sb[:])

    nc.compile()
    return nc


def fold_inputs(inputs):
    """Host-side weight folding. Returns the per-core-common input dict."""
    f32 = np.float32
    g = {k: np.asarray(v, f32) for k, v in inputs.items()}
    Wk, Wr, lb = g["lstm_k"], g["lstm_r"], g["lstm_b"]

    blk0 = np.arange(2 * U)            # (i, f)
    blk1 = 2 * U + np.arange(2 * U)    # (g, o)
    gsc = np.concatenate([2 * np.ones(U, f32), np.ones(U, f32)])

    sl = g["bnl_g"] / np.sqrt(g["bnl_v"] + EPS)
    tl = g["bnl_b"] - g["bnl_m"] * sl
    g1s = g["bn1_g"] / np.sqrt(g["bn1_v"] + EPS)
    d1s = g["bn1_b"] - g["bn1_m"] * g1s
    g2s = g["bn2_g"] / np.sqrt(g["bn2_v"] + EPS)
    d2s = g["bn2_b"] - g["bn2_m"] * g2s

    # structural requirements of the rank-1 GCN collapse
    assert np.abs(g["b1"]).max() == 0.0, "kernel requires b1 == 0"
    assert np.abs(d1s @ g["w2"]).max() < 1e-30, "kernel requires bn1 shift @ w2 == 0"
    assert np.abs(g["b2"]).max() == 0.0, "kernel requires b2 == 0"
    assert (g2s > 0).all(), "kernel requires positive bn2 scale"

    w2pp = (g1s[:, None] * g["w2"]) * g2s[None, :]
    D4 = g["d1_w"].reshape(T, N, K2, 3)
    constp = np.einsum("m,tnmp->p", d2s, D4)
    c1v = tl @ g["w1"]

    vals = {
        "ident": np.eye(128, dtype=f32),
        "wk0": Wk[:, blk0], "wk1": Wk[:, blk1],
        "wr0": Wr[:, blk0], "wr1": Wr[:, blk1],
        "w1p": sl[:, None] * g["w1"],
        "w2rep": np.repeat(w2pp, 3, axis=1),
        "d2w": g["d2_w"],
        "sel96": np.kron(np.ones((K2, 1), f32), np.eye(3, dtype=f32)),
        "d2b": g["d2_b"].reshape(1, N),
        "ones14": np.ones((1, BL), f32),
        "bias0": lb[blk0].reshape(128, 1),
        "bias1": (gsc * lb[blk1]).reshape(128, 1),
        "scale1": np.concatenate(
            [2 * np.ones(U, f32), np.ones(U, f32)]).reshape(128, 1),
        "c1": c1v.reshape(K1, 1),
        "c1n": (-c1v).reshape(K1, 1),
        "ones128": np.ones((128, 1), f32),
        "d1b3": (g["d1_b"] + constp).reshape(3, 1),
    }
    cb = np.zeros((128, BUNDLE_W), f32)
    for name, (off, rows, cols) in _BUNDLE.items():
        v = vals[name]
        assert v.shape == (rows, cols), (name, v.shape, (rows, cols))
        cb[0:rows, off:off + cols] = v
    return {"cbundle": cb, "d1w": np.ascontiguousarray(g["d1_w"])}


def make_in_maps(inputs):
    common = fold_inputs(inputs)
    x = np.asarray(inputs["x"], np.float32)
    a = np.asarray(inputs["a"], np.float32)
    in_maps = []
    for core in range(NCORE):
        m = dict(common)
        m["x_sh"] = np.ascontiguousarray(
            x[core * BL:(core + 1) * BL].reshape(BL * T, F))
        m["a_sh"] = np.ascontiguousarray(a[core * BL:(core + 1) * BL])
        in_maps.append(m)
    return in_maps


def kernel(**inputs):
    from concourse.bass_utils import run_bass_kernel_spmd

    if "module" not in _CACHE:
        _CACHE["module"] = build_module()
    nc = _CACHE["module"]

    in_maps = make_in_maps(inputs)
    res = run_bass_kernel_spmd(nc, in_maps, core_ids=list(range(NCORE)))
    out = np.concatenate([res.results[i]["out_sh"] for i in range(NCORE)], axis=0)
    return out.astype(np.float32)


# revision 21
# speedup vs baseline: 1.8000x; 1.1437x over previous
"""Trainium2 Bass kernel for nn_EndToEndCryptoModel (LSTM -> GCNx2 -> Dense).

Strategy (per-core, data-parallel over batch, 4 batches/core on 8 cores):
  * LSTM solved by Picard fixed-point iteration over the whole sequence:
    7 iterations, each fully parallel over (b, t) using big sigmoid ops
    (all 4 gates via sigmoid; tanh(y) = 2*sigmoid(2y)-1 with the 2x folded
    into weights / per-partition ACT scale), and the cell-state recurrence
    done by a single DVE tensor_tensor_scan along the time axis (batch
    chains separated by poison pad columns that reset the scan).
  * The GCN collapses algebraically: sup1 is node-independent, so
    g1 = leaky(rowsum(a) (x) s1) is rank-1 (leaky is positively homogeneous
    and b1 == 0), and the whole two-layer GCN reduces to per-(t,m) scalars
    q[t,m] and per-node weights w[n] = (a @ (a @ 1))[n].
  * Final dense layer: d1[b,p] = sum_{t,m} Lq'[b,t,m] * (w[b,:] @ D[t,:,m,p]),
    computed as 64 small matmuls with D t-slices as stationary weights into a
    [(m,p)=96, (t,b)] PSUM layout that exactly matches the layout q comes in
    (w2 columns pre-replicated x3 on the host), then DVE multiply+reduce and
    two tiny matmuls.

All heavy layout decisions are hardcoded for the fixed problem shapes.
"""

import numpy as np

B, T, N, F = 32, 64, 128, 128
U, K1, K2 = 64, 64, 32
NCORE = 8
BL = B // NCORE            # 4 batches per core
CW = BL * (T + 1)          # 260 columns, b-major with pad col at b*(T+1)
NEG = -1e30
EPS = 1e-3
SLOPE = 0.01
N_ITERS = 6

# dtype configuration for the fast matmul paths
FAST_Z = True
FAST_M1 = "f32r"

_CACHE = {}

# constant-bundle column layout: name -> (col_off, rows, cols)
_BUNDLE = {}
_off = 0
for _name, _rows, _cols in [
    ("ident", 128, 128), ("wk0", 128, 128), ("wk1", 128, 128),
    ("wr0", 64, 128), ("wr1", 64, 128), ("w1p", 64, 64), ("w2rep", 64, 96),
    ("d2w", 3, 128), ("sel96", 96, 3), ("d2b", 1, 128), ("ones14", 1, 4),
    ("bias0", 128, 1), ("bias1", 128, 1), ("scale1", 128, 1),
    ("c1", 64, 1), ("c1n", 64, 1), ("ones128", 128, 1), ("d1b3", 3, 1),
]:
    _BUNDLE[_name] = (_off, _rows, _cols)
    _off += _cols
BUNDLE_W = _off


def build_module(fast_z=False, fast_m1="f32"):
    """Build the per-core Bass/Tile module (identical SPMD program).

    fast_z: use float32r for the per-iteration z matmuls (xz + Wr@h).
    fast_m1: use float32r for the M1T (dense-layer) matmuls.
    """
    from contextlib import ExitStack
    import concourse.bacc as bacc
    import concourse.mybir as mybir
    from concourse import tile

    f32 = mybir.dt.float32
    f32r = mybir.dt.float32r
    bf16 = mybir.dt.bfloat16
    zdt = f32r if fast_z else f32
    mdt = {"f32": f32, "f32r": f32r, "bf16": bf16}[fast_m1]
    Alu = mybir.AluOpType
    Act = mybir.ActivationFunctionType

    nc = bacc.Bacc(None, target_bir_lowering=False)

    # ---------------- DRAM I/O ----------------
    x_d = nc.dram_tensor("x_sh", [BL * T, F], f32, kind="ExternalInput")
    a_d = nc.dram_tensor("a_sh", [BL, N, N], f32, kind="ExternalInput")
    cb_d = nc.dram_tensor("cbundle", [128, BUNDLE_W], f32, kind="ExternalInput")
    d1w_d = nc.dram_tensor("d1w", [T * N * K2, 3], mdt, kind="ExternalInput")
    out_d = nc.dram_tensor("out_sh", [BL, N], f32, kind="ExternalOutput")

    with tile.TileContext(nc) as tc, ExitStack() as ctx:
        cp = ctx.enter_context(tc.tile_pool(name="const", bufs=1))
        wp = ctx.enter_context(tc.tile_pool(name="work", bufs=2))
        pz = ctx.enter_context(tc.tile_pool(name="pz", bufs=1, space="PSUM"))
        pm = ctx.enter_context(tc.tile_pool(name="pm", bufs=2, space="PSUM"))
        pt = ctx.enter_context(tc.tile_pool(name="pt", bufs=2, space="PSUM"))
        ps = ctx.enter_context(tc.tile_pool(name="ps", bufs=2, space="PSUM"))

        dma = nc.sync.dma_start

        # ---- DMAs: constants bundle + x first, then a; big D last ----
        cb = cp.tile([128, BUNDLE_W], f32, tag="cb")
        dma(cb[:], cb_d[:])

        def cview(name):
            off, rows, cols = _BUNDLE[name]
            return cb[0:rows, off:off + cols]

        x2 = wp.tile([128, 256], f32, tag="x2")
        dma(x2[:].rearrange("p (i f) -> p i f", i=2),
            x_d[:].rearrange("(i p) f -> p i f", i=2))

        a_all = wp.tile([128, BL * N], f32, tag="a_all")
        dma(a_all[:].rearrange("p (b n) -> p b n", b=BL),
            a_d[:].rearrange("b p n -> p b n"))

        D_sb = cp.tile([128, T * K2 * 3], mdt, tag="Dsb")
        d1w_view = d1w_d[:].rearrange("(t n m) p -> n t (m p)", t=T, n=N, m=K2)
        dma(D_sb[:].rearrange("n (t mp) -> n t mp", t=T), d1w_view)

        ident = cview("ident")
        if fast_z:
            identr = cp.tile([128, 128], f32r, tag="identr")
            nc.vector.tensor_copy(identr[:], ident)
            ident_z = identr[:]
        else:
            ident_z = ident

        # ---- x transpose + xz precompute ----
        xt_sb = []
        for i in range(2):
            tp = pt.tile([128, 128], f32, tag="tp")
            nc.tensor.transpose(tp[:], x2[:, i * 128:(i + 1) * 128], ident)
            xt = cp.tile([128, 128], f32, tag=f"xt{i}")
            nc.vector.tensor_copy(xt[:], tp[:])
            xt_sb.append(xt)

        # xz[blk] = Wk_blk.T @ xT  scattered to b-major pad layout [128, 260]
        xzt = []
        for blk in range(2):
            wk = cview("wk0" if blk == 0 else "wk1")
            xzp = pt.tile([128, CW], f32, tag="tp")
            xzp3 = xzp[:].rearrange("p (b t) -> p b t", b=BL)
            for b in range(BL):
                i, bl = divmod(b, 2)
                nc.tensor.matmul(
                    xzp[:, b * (T + 1) + 1:b * (T + 1) + 1 + T],
                    wk[:], xt_sb[i][:, bl * T:(bl + 1) * T],
                    start=True, stop=True,
                )
            xz_sb = cp.tile([128, CW], zdt, tag=f"xzt{blk}")
            xz3 = xz_sb[:].rearrange("p (b t) -> p b t", b=BL)
            nc.vector.tensor_copy(xz3[:, :, 1:T + 1], xzp3[:, :, 1:T + 1])
            nc.vector.memset(xz3[:, :, 0:1].bitcast(f32), NEG)
            xzt.append(xz_sb)

        # wcols [128 (n), 4 (b)] filled by A-prep inside iteration 0
        wcols = cp.tile([128, BL], mdt, tag="wcols")
        ones128 = cview("ones128")

        # M1T psum banks: [(m,p)=96, (t-local, b)=128] x2, filled by 64
        # per-t matmuls (lhsT = D t-slice) interleaved into the LSTM
        m1t = [
            pm.tile([96, 32 * BL], f32, tag="m1t", name=f"m1t{i}")
            for i in range(2)
        ]

        # ---- LSTM Picard iterations ----
        bias0, bias1, scale1 = cview("bias0"), cview("bias1"), cview("scale1")
        wr0z = cp.tile([U, 128], zdt, tag="wr0z")
        nc.vector.tensor_copy(wr0z[:], cview("wr0"))
        wr1z = cp.tile([U, 128], zdt, tag="wr1z")
        nc.vector.tensor_copy(wr1z[:], cview("wr1"))

        h = None
        m1_sched = {1: 13, 2: 13, 3: 13, 4: 13, 5: 12}
        m1_done = 0

        for it in range(N_ITERS):
            # z1 (g,o block) first: its sigmoid leads the critical chain
            zp = {}
            for blk, wr, xz_sb in ((1, wr1z, xzt[1]), (0, wr0z, xzt[0])):
                z = pz.tile([128, CW], f32, tag=f"z{blk}", name=f"z{blk}_{it}")
                if it == 0:
                    nc.tensor.matmul(z[:], ident_z, xz_sb[:],
                                     start=True, stop=True)
                else:
                    nc.tensor.matmul(z[:], ident_z, xz_sb[:],
                                     start=True, stop=False)
                    nc.tensor.matmul(z[:], wr[:], h[:, 0:CW],
                                     start=False, stop=True)
                zp[blk] = z
            # s1 = sigmoid(scale1*z_go + b_go'): Sg'=sig(2 z_g) rows 0:64,
            # So rows 64:128
            s1 = wp.tile([128, CW], f32, tag="s1")
            nc.scalar.activation(s1[:], zp[1][:], Act.Sigmoid,
                                 bias=bias1, scale=scale1)
            # s0 = sigmoid(z_if + b_if): Si rows 0:64, Sf rows 64:128
            s0 = wp.tile([128, CW], f32, tag="s0")
            nc.scalar.activation(s0[:], zp[0][:], Act.Sigmoid,
                                 bias=bias0, scale=1.0)
            # g2 = tanh(z_g) = 2*Sg' - 1  (runs on DVE while s0 is on ACT)
            g2 = wp.tile([U, CW], f32, tag="g2")
            nc.vector.tensor_scalar(g2[:], s1[0:U], 2.0, 1.0,
                                    Alu.mult, Alu.subtract)
            # v = i*g = Si*g2, written at base partition 64 so the scan's
            # two inputs (Sf, v) share a base
            v = wp.tile([128, CW], f32, tag="v")
            nc.vector.tensor_tensor(v[U:128], s0[0:U], g2[:], Alu.mult)
            c = wp.tile([128, CW], f32, tag="c")
            nc.vector.tensor_tensor_scan(
                c[U:128], s0[U:128], v[U:128], 0.0, Alu.mult, Alu.add)
            th = wp.tile([128, CW], f32, tag="th")
            nc.scalar.activation(th[U:128], c[U:128], Act.Tanh,
                                 bias=0.0, scale=1.0)
            h = wp.tile([U, CW + 1], zdt, tag="h")
            nc.vector.tensor_tensor(h[:, 1:CW + 1], s1[U:128], th[U:128],
                                    Alu.mult)
            nc.vector.memset(h[:, 0:1].bitcast(f32), 0.0)

            if it == 0:
                # A-prep fills the PE/DVE gap while iteration 0's sigmoid
                # chain runs: AT, r = A@1, w = A@r -> wcols
                for b in range(BL):
                    tp = pt.tile([128, 128], f32, tag="tp")
                    nc.tensor.transpose(tp[:], a_all[:, b * N:(b + 1) * N],
                                        ident)
                    at_sb = wp.tile([128, N], f32, tag="atsb")
                    nc.vector.tensor_copy(at_sb[:], tp[:])
                    rp = ps.tile([128, 1], f32, tag="small")
                    nc.tensor.matmul(rp[:], at_sb[:], ones128,
                                     start=True, stop=True)
                    r_sb = wp.tile([128, 1], f32, tag="rsb")
                    nc.vector.tensor_copy(r_sb[:], rp[:])
                    wpm = ps.tile([128, 1], f32, tag="small")
                    nc.tensor.matmul(wpm[:], at_sb[:], r_sb[:],
                                     start=True, stop=True)
                    nc.vector.tensor_copy(wcols[:, b:b + 1], wpm[:])

            # M1T matmuls (need D + wcols only), hidden inside iterations
            for t in range(m1_done, m1_done + m1_sched.get(it, 0)):
                bank, tl = t // 32, t % 32
                nc.tensor.matmul(
                    m1t[bank][:, tl * BL:(tl + 1) * BL],
                    D_sb[:, t * 96:(t + 1) * 96], wcols[:],
                    start=True, stop=True,
                )
            m1_done += m1_sched.get(it, 0)

        # ---- GCN tail ----
        if fast_z:
            w1pz = cp.tile([U, K1], zdt, tag="w1pz")
            nc.vector.tensor_copy(w1pz[:], cview("w1p"))
            w1p_use = w1pz[:]
        else:
            w1p_use = cview("w1p")
        s1p = pt.tile([K1, CW], f32, tag="tp")
        nc.tensor.matmul(s1p[:], w1p_use, h[:, 1:CW + 1],
                         start=True, stop=True)
        # leaky(y) = y + (1-slope)*relu(-y), y = s1 + c1
        rn1 = wp.tile([K1, CW], f32, tag="rn1")
        nc.scalar.activation(rn1[:], s1p[:], Act.Relu,
                             bias=cview("c1n"), scale=-1.0)
        L1a = wp.tile([K1, CW], f32, tag="L1a")
        nc.vector.scalar_tensor_tensor(
            L1a[:], rn1[:], 1.0 - SLOPE, s1p[:], Alu.mult, Alu.add)
        L1 = wp.tile([K1, CW], f32, tag="L1")
        nc.vector.tensor_scalar_add(L1[:], L1a[:], cview("c1"))
        qp = pt.tile([96, CW], f32, tag="tp")
        nc.tensor.matmul(qp[:], cview("w2rep"), L1[:], start=True, stop=True)
        rn2 = wp.tile([96, CW], f32, tag="rn2")
        nc.scalar.activation(rn2[:], qp[:], Act.Relu, bias=0.0, scale=-1.0)
        lq = wp.tile([96, CW], f32, tag="lq")
        nc.vector.scalar_tensor_tensor(
            lq[:], rn2[:], 1.0 - SLOPE, qp[:], Alu.mult, Alu.add)

        # dsum[(m,p), b] = sum_t lq[(m,p), (b,t)] * m1t[(m,p), (t,b)]
        lqv = lq[:].rearrange("p (b t) -> p t b", b=BL)      # [96, 65, 4]
        dparts = []
        for bank in range(2):
            prod = wp.tile([96, 32 * BL], f32, tag="prod", name=f"prod{bank}")
            pv = prod[:].rearrange("p (t b) -> p t b", b=BL)  # [96, 32, 4]
            nc.vector.tensor_tensor(
                pv[:], lqv[:, 1 + bank * 32:1 + bank * 32 + 32, :],
                m1t[bank][:].rearrange("p (t b) -> p t b", b=BL), Alu.mult)
            dp = wp.tile([96, BL], f32, tag="dpart", name=f"dpart{bank}")
            nc.vector.tensor_reduce(
                dp[:], prod[:].rearrange("p (t b) -> p b t", b=BL),
                mybir.AxisListType.X, Alu.add)
            dparts.append(dp)
        dsum = wp.tile([96, BL], f32, tag="dsum")
        nc.vector.tensor_tensor(dsum[:], dparts[0][:], dparts[1][:], Alu.add)

        d1p = ps.tile([3, BL], f32, tag="small")
        nc.tensor.matmul(d1p[:], cview("sel96"), dsum[:], start=True, stop=True)
        d1r = wp.tile([3, BL], f32, tag="d1r")
        nc.scalar.activation(d1r[:], d1p[:], Act.Relu,
                             bias=cview("d1b3"), scale=1.0)

        op = ps.tile([BL, N], f32, tag="small")
        nc.tensor.matmul(op[:], d1r[:], cview("d2w"), start=True, stop=False)
        nc.tensor.matmul(op[:], cview("ones14"), cview("d2b"),
                         start=False, stop=True)
        out_sb = wp.tile([BL, N], f32, tag="outsb")
        nc.scalar.copy(out_sb[:], op[:])
        dma(out_d[:], out_sb[:])

    nc.compile()
    return nc


def fold_inputs(inputs):
    """Host-side weight folding. Returns the per-core-common input dict."""
    f32 = np.float32
    g = {k: np.asarray(v, f32) for k, v in inputs.items()}
    Wk, Wr, lb = g["lstm_k"], g["lstm_r"], g["lstm_b"]

    blk0 = np.arange(2 * U)            # (i, f)
    blk1 = 2 * U + np.arange(2 * U)    # (g, o)
    gsc = np.concatenate([2 * np.ones(U, f32), np.ones(U, f32)])

    sl = g["bnl_g"] / np.sqrt(g["bnl_v"] + EPS)
    tl = g["bnl_b"] - g["bnl_m"] * sl
    g1s = g["bn1_g"] / np.sqrt(g["bn1_v"] + EPS)
    d1s = g["bn1_b"] - g["bn1_m"] * g1s
    g2s = g["bn2_g"] / np.sqrt(g["bn2_v"] + EPS)
    d2s = g["bn2_b"] - g["bn2_m"] * g2s

    # structural requirements of the rank-1 GCN collapse
    assert np.abs(g["b1"]).max() == 0.0, "kernel requires b1 == 0"
    assert np.abs(d1s @ g["w2"]).max() < 1e-30, "kernel requires bn1 shift @ w2 == 0"
    assert np.abs(g["b2"]).max() == 0.0, "kernel requires b2 == 0"
    assert (g2s > 0).all(), "kernel requires positive bn2 scale"

    w2pp = (g1s[:, None] * g["w2"]) * g2s[None, :]
    D4 = g["d1_w"].reshape(T, N, K2, 3)
    constp = np.einsum("m,tnmp->p", d2s, D4)
    c1v = tl @ g["w1"]

    vals = {
        "ident": np.eye(128, dtype=f32),
        "wk0": Wk[:, blk0], "wk1": Wk[:, blk1],
        "wr0": Wr[:, blk0], "wr1": Wr[:, blk1],
        "w1p": sl[:, None] * g["w1"],
        "w2rep": np.repeat(w2pp, 3, axis=1),
        "d2w": g["d2_w"],
        "sel96": np.kron(np.ones((K2, 1), f32), np.eye(3, dtype=f32)),
        "d2b": g["d2_b"].reshape(1, N),
        "ones14": np.ones((1, BL), f32),
        "bias0": lb[blk0].reshape(128, 1),
        "bias1": (gsc * lb[blk1]).reshape(128, 1),
        "scale1": np.concatenate(
            [2 * np.ones(U, f32), np.ones(U, f32)]).reshape(128, 1),
        "c1": c1v.reshape(K1, 1),
        "c1n": (-c1v).reshape(K1, 1),
        "ones128": np.ones((128, 1), f32),
        "d1b3": (g["d1_b"] + constp).reshape(3, 1),
    }
    cb = np.zeros((128, BUNDLE_W), f32)
    for name, (off, rows, cols) in _BUNDLE.items():
        v = vals[name]
        assert v.shape == (rows, cols), (name, v.shape, (rows, cols))
        cb[0:rows, off:off + cols] = v
    return {"cbundle": cb, "d1w": np.ascontiguousarray(g["d1_w"])}


def _cast_d1w(arr, fast_m1):
    if fast_m1 == "bf16":
        import ml_dtypes
        return np.ascontiguousarray(arr.astype(ml_dtypes.bfloat16))
    return arr


def make_in_maps(inputs, fast_m1="f32"):
    common = fold_inputs(inputs)
    common["d1w"] = _cast_d1w(common["d1w"], fast_m1)
    x = np.asarray(inputs["x"], np.float32)
    a = np.asarray(inputs["a"], np.float32)
    in_maps = []
    for core in range(NCORE):
        m = dict(common)
        m["x_sh"] = np.ascontiguousarray(
            x[core * BL:(core + 1) * BL].reshape(BL * T, F))
        m["a_sh"] = np.ascontiguousarray(a[core * BL:(core + 1) * BL])
        in_maps.append(m)
    return in_maps


def kernel(**inputs):
    from concourse.bass_utils import run_bass_kernel_spmd

    if "module" not in _CACHE:
        _CACHE["module"] = build_module(fast_z=FAST_Z, fast_m1=FAST_M1)
    nc = _CACHE["module"]

    in_maps = make_in_maps(inputs, FAST_M1)
    res = run_bass_kernel_spmd(nc, in_maps, core_ids=list(range(NCORE)))
    out = np.concatenate([res.results[i]["out_sh"] for i in range(NCORE)], axis=0)
    return out.astype(np.float32)


# revision 23
# speedup vs baseline: 1.8665x; 1.0369x over previous
"""Trainium2 Bass kernel for nn_EndToEndCryptoModel (LSTM -> GCNx2 -> Dense).

Strategy (per-core, data-parallel over batch, 4 batches/core on 8 cores):
  * LSTM solved by Picard fixed-point iteration over the whole sequence:
    7 iterations, each fully parallel over (b, t) using big sigmoid ops
    (all 4 gates via sigmoid; tanh(y) = 2*sigmoid(2y)-1 with the 2x folded
    into weights / per-partition ACT scale), and the cell-state recurrence
    done by a single DVE tensor_tensor_scan along the time axis (batch
    chains separated by poison pad columns that reset the scan).
  * The GCN collapses algebraically: sup1 is node-independent, so
    g1 = leaky(rowsum(a) (x) s1) is rank-1 (leaky is positively homogeneous
    and b1 == 0), and the whole two-layer GCN reduces to per-(t,m) scalars
    q[t,m] and per-node weights w[n] = (a @ (a @ 1))[n].
  * Final dense layer: d1[b,p] = sum_{t,m} Lq'[b,t,m] * (w[b,:] @ D[t,:,m,p]),
    computed as 64 small matmuls with D t-slices as stationary weights into a
    [(m,p)=96, (t,b)] PSUM layout that exactly matches the layout q comes in
    (w2 columns pre-replicated x3 on the host), then DVE multiply+reduce and
    two tiny matmuls.

All heavy layout decisions are hardcoded for the fixed problem shapes.
"""

import numpy as np

B, T, N, F = 32, 64, 128, 128
U, K1, K2 = 64, 64, 32
NCORE = 8
BL = B // NCORE            # 4 batches per core
CW = BL * (T + 1)          # 260 columns, b-major with pad col at b*(T+1)
NEG = -1e30
EPS = 1e-3
SLOPE = 0.01
N_ITERS = 6

# dtype configuration for the fast matmul paths
FAST_Z = True
FAST_M1 = "f32r"

_CACHE = {}

# constant-bundle column layout: name -> (col_off, rows, cols)
_BUNDLE = {}
_off = 0
for _name, _rows, _cols in [
    ("ident", 128, 128), ("wk0", 128, 128), ("wk1", 128, 128),
    ("wr0", 64, 128), ("wr1", 64, 128), ("w1p", 64, 64), ("w2rep", 64, 96),
    ("d2w", 3, 128), ("sel96", 96, 3), ("d2b", 1, 128), ("ones14", 1, 4),
    ("bias0", 128, 1), ("bias1", 128, 1), ("scale1", 128, 1),
    ("c1", 64, 1), ("c1n", 64, 1), ("ones128", 128, 1), ("d1b3", 3, 1),
]:
    _BUNDLE[_name] = (_off, _rows, _cols)
    _off += _cols
BUNDLE_W = _off


def build_module(fast_z=False, fast_m1="f32"):
    """Build the per-core Bass/Tile module (identical SPMD program).

    fast_z: use float32r for the per-iteration z matmuls (xz + Wr@h).
    fast_m1: use float32r for the M1T (dense-layer) matmuls.
    """
    from contextlib import ExitStack
    import concourse.bacc as bacc
    import concourse.mybir as mybir
    from concourse import tile

    f32 = mybir.dt.float32
    f32r = mybir.dt.float32r
    bf16 = mybir.dt.bfloat16
    zdt = f32r if fast_z else f32
    mdt = {"f32": f32, "f32r": f32r, "bf16": bf16}[fast_m1]
    Alu = mybir.AluOpType
    Act = mybir.ActivationFunctionType

    nc = bacc.Bacc(None, target_bir_lowering=False)

    # ---------------- DRAM I/O ----------------
    x_d = nc.dram_tensor("x_sh", [BL * T, F], f32, kind="ExternalInput")
    a_d = nc.dram_tensor("a_sh", [BL, N, N], f32, kind="ExternalInput")
    cb_d = nc.dram_tensor("cbundle", [128, BUNDLE_W], f32, kind="ExternalInput")
    d1w_d = nc.dram_tensor("d1w", [T * N * K2, 3], mdt, kind="ExternalInput")
    out_d = nc.dram_tensor("out_sh", [BL, N], f32, kind="ExternalOutput")

    with tile.TileContext(nc) as tc, ExitStack() as ctx:
        cp = ctx.enter_context(tc.tile_pool(name="const", bufs=1))
        wp = ctx.enter_context(tc.tile_pool(name="work", bufs=2))
        pz = ctx.enter_context(tc.tile_pool(name="pz", bufs=1, space="PSUM"))
        pm = ctx.enter_context(tc.tile_pool(name="pm", bufs=2, space="PSUM"))
        pt = ctx.enter_context(tc.tile_pool(name="pt", bufs=2, space="PSUM"))
        ps = ctx.enter_context(tc.tile_pool(name="ps", bufs=2, space="PSUM"))

        dma = nc.sync.dma_start

        # ---- DMAs: constants bundle + x first, then a; big D last ----
        cb = cp.tile([128, BUNDLE_W], f32, tag="cb")
        dma(cb[:], cb_d[:])

        def cview(name):
            off, rows, cols = _BUNDLE[name]
            return cb[0:rows, off:off + cols]

        x2 = wp.tile([128, 256], f32, tag="x2")
        dma(x2[:].rearrange("p (i f) -> p i f", i=2),
            x_d[:].rearrange("(i p) f -> p i f", i=2))

        a_all = wp.tile([128, BL * N], f32, tag="a_all")
        dma(a_all[:].rearrange("p (b n) -> p b n", b=BL),
            a_d[:].rearrange("b p n -> p b n"))

        D_sb = cp.tile([128, T * K2 * 3], mdt, tag="Dsb")
        d1w_view = d1w_d[:].rearrange("(t n m) p -> n t (m p)", t=T, n=N, m=K2)
        dma(D_sb[:].rearrange("n (t mp) -> n t mp", t=T), d1w_view)

        ident = cview("ident")
        if fast_z:
            identr = cp.tile([128, 128], f32r, tag="identr")
            nc.vector.tensor_copy(identr[:], ident)
            ident_z = identr[:]
        else:
            ident_z = ident

        # ---- x transpose + xz precompute ----
        xt_sb = []
        for i in range(2):
            tp = pt.tile([128, 128], f32, tag="tp")
            nc.tensor.transpose(tp[:], x2[:, i * 128:(i + 1) * 128], ident)
            xt = cp.tile([128, 128], f32, tag=f"xt{i}")
            nc.vector.tensor_copy(xt[:], tp[:])
            xt_sb.append(xt)

        # xz[blk] = Wk_blk.T @ xT  scattered to b-major pad layout
        # [128, 260], written directly into iteration 0's z psum banks.
        # SBUF copies (used by iterations 1+) evacuate on ACT off-path.
        xzt = []
        z_it0 = []
        for blk in (1, 0):
            wk = cview("wk0" if blk == 0 else "wk1")
            xzp = pz.tile([128, CW], f32, tag=f"z{blk}", name=f"z{blk}_xz")
            xzp3 = xzp[:].rearrange("p (b t) -> p b t", b=BL)
            for b in range(BL):
                i, bl = divmod(b, 2)
                nc.tensor.matmul(
                    xzp[:, b * (T + 1) + 1:b * (T + 1) + 1 + T],
                    wk[:], xt_sb[i][:, bl * T:(bl + 1) * T],
                    start=True, stop=True,
                )
            nc.vector.memset(xzp3[:, :, 0:1], NEG)
            xz_sb = cp.tile([128, CW], zdt, tag=f"xzt{blk}")
            xz3 = xz_sb[:].rearrange("p (b t) -> p b t", b=BL)
            nc.scalar.copy(xz3[:, :, 1:T + 1], xzp3[:, :, 1:T + 1])
            nc.vector.memset(xz3[:, :, 0:1].bitcast(f32), NEG)
            z_it0.append(xzp)
            xzt.append(xz_sb)
        z_it0 = {1: z_it0[0], 0: z_it0[1]}
        xzt = {1: xzt[0], 0: xzt[1]}
        xzt = [xzt[0], xzt[1]]

        # wcols [128 (n), 4 (b)] filled by A-prep inside iteration 0
        wcols = cp.tile([128, BL], mdt, tag="wcols")
        ones128 = cview("ones128")

        # M1T psum banks: [(m,p)=96, (t-local, b)=128] x2, filled by 64
        # per-t matmuls (lhsT = D t-slice) interleaved into the LSTM
        m1t = [
            pm.tile([96, 32 * BL], f32, tag="m1t", name=f"m1t{i}")
            for i in range(2)
        ]

        # ---- LSTM Picard iterations ----
        bias0, bias1, scale1 = cview("bias0"), cview("bias1"), cview("scale1")
        wr0z = cp.tile([U, 128], zdt, tag="wr0z")
        nc.vector.tensor_copy(wr0z[:], cview("wr0"))
        wr1z = cp.tile([U, 128], zdt, tag="wr1z")
        nc.vector.tensor_copy(wr1z[:], cview("wr1"))

        h = None
        m1_sched = {1: 13, 2: 13, 3: 13, 4: 13, 5: 12}
        m1_done = 0

        for it in range(N_ITERS):
            # z1 (g,o block) first: its sigmoid leads the critical chain
            zp = {}
            if it == 0:
                zp = z_it0
            else:
                for blk, wr, xz_sb in ((1, wr1z, xzt[1]), (0, wr0z, xzt[0])):
                    z = pz.tile([128, CW], f32, tag=f"z{blk}",
                                name=f"z{blk}_{it}")
                    nc.tensor.matmul(z[:], ident_z, xz_sb[:],
                                     start=True, stop=False)
                    nc.tensor.matmul(z[:], wr[:], h[:, 0:CW],
                                     start=False, stop=True)
                    zp[blk] = z
            # s1 = sigmoid(scale1*z_go + b_go'): Sg'=sig(2 z_g) rows 0:64,
            # So rows 64:128
            s1 = wp.tile([128, CW], f32, tag="s1")
            nc.scalar.activation(s1[:], zp[1][:], Act.Sigmoid,
                                 bias=bias1, scale=1.0)
            # s0 = sigmoid(z_if + b_if): Si rows 0:64, Sf rows 64:128
            s0 = wp.tile([128, CW], f32, tag="s0")
            nc.scalar.activation(s0[:], zp[0][:], Act.Sigmoid,
                                 bias=bias0, scale=1.0)
            # g2 = tanh(z_g) = 2*Sg' - 1  (runs on DVE while s0 is on ACT)
            g2 = wp.tile([U, CW], f32, tag="g2")
            nc.vector.tensor_scalar(g2[:], s1[0:U], 2.0, 1.0,
                                    Alu.mult, Alu.subtract)
            # v = i*g = Si*g2, written at base partition 64 so the scan's
            # two inputs (Sf, v) share a base
            v = wp.tile([128, CW], f32, tag="v")
            nc.vector.tensor_tensor(v[U:128], s0[0:U], g2[:], Alu.mult)
            c = wp.tile([128, CW], f32, tag="c")
            nc.vector.tensor_tensor_scan(
                c[U:128], s0[U:128], v[U:128], 0.0, Alu.mult, Alu.add)
            th = wp.tile([128, CW], f32, tag="th")
            nc.scalar.activation(th[U:128], c[U:128], Act.Tanh,
                                 bias=0.0, scale=1.0)
            h = wp.tile([U, CW + 1], zdt, tag="h")
            nc.vector.tensor_tensor(h[:, 1:CW + 1], s1[U:128], th[U:128],
                                    Alu.mult)
            nc.vector.memset(h[:, 0:1].bitcast(f32), 0.0)

            if it == 0:
                # A-prep fills the PE/DVE gap while iteration 0's sigmoid
                # chain runs: AT, r = A@1, w = A@r -> wcols
                for b in range(BL):
                    tp = pt.tile([128, 128], f32, tag="tp")
                    nc.tensor.transpose(tp[:], a_all[:, b * N:(b + 1) * N],
                                        ident)
                    at_sb = wp.tile([128, N], f32, tag="atsb")
                    nc.vector.tensor_copy(at_sb[:], tp[:])
                    rp = ps.tile([128, 1], f32, tag="small")
                    nc.tensor.matmul(rp[:], at_sb[:], ones128,
                                     start=True, stop=True)
                    r_sb = wp.tile([128, 1], f32, tag="rsb")
                    nc.vector.tensor_copy(r_sb[:], rp[:])
                    wpm = ps.tile([128, 1], f32, tag="small")
                    nc.tensor.matmul(wpm[:], at_sb[:], r_sb[:],
                                     start=True, stop=True)
                    nc.vector.tensor_copy(wcols[:, b:b + 1], wpm[:])

            # M1T matmuls (need D + wcols only), hidden inside iterations
            for t in range(m1_done, m1_done + m1_sched.get(it, 0)):
                bank, tl = t // 32, t % 32
                nc.tensor.matmul(
                    m1t[bank][:, tl * BL:(tl + 1) * BL],
                    D_sb[:, t * 96:(t + 1) * 96], wcols[:],
                    start=True, stop=True,
                )
            m1_done += m1_sched.get(it, 0)

        # ---- GCN tail ----
        if fast_z:
            w1pz = cp.tile([U, K1], zdt, tag="w1pz")
            nc.vector.tensor_copy(w1pz[:], cview("w1p"))
            w1p_use = w1pz[:]
        else:
            w1p_use = cview("w1p")
        s1p = pt.tile([K1, CW], f32, tag="tp")
        nc.tensor.matmul(s1p[:], w1p_use, h[:, 1:CW + 1],
                         start=True, stop=True)
        # leaky(y) = y + (1-slope)*relu(-y), y = s1 + c1
        rn1 = wp.tile([K1, CW], f32, tag="rn1")
        nc.scalar.activation(rn1[:], s1p[:], Act.Relu,
                             bias=cview("c1n"), scale=-1.0)
        L1a = wp.tile([K1, CW], f32, tag="L1a")
        nc.vector.scalar_tensor_tensor(
            L1a[:], rn1[:], 1.0 - SLOPE, s1p[:], Alu.mult, Alu.add)
        L1 = wp.tile([K1, CW], zdt, tag="L1")
        nc.vector.tensor_scalar_add(L1[:], L1a[:], cview("c1"))
        if fast_z:
            w2z = cp.tile([K1, 96], zdt, tag="w2z")
            nc.vector.tensor_copy(w2z[:], cview("w2rep"))
            w2_use = w2z[:]
        else:
            w2_use = cview("w2rep")
        qp = pt.tile([96, CW], f32, tag="tp")
        nc.tensor.matmul(qp[:], w2_use, L1[:], start=True, stop=True)
        rn2 = wp.tile([96, CW], f32, tag="rn2")
        nc.scalar.activation(rn2[:], qp[:], Act.Relu, bias=0.0, scale=-1.0)
        lq = wp.tile([96, CW], f32, tag="lq")
        nc.vector.scalar_tensor_tensor(
            lq[:], rn2[:], 1.0 - SLOPE, qp[:], Alu.mult, Alu.add)

        # dsum[(m,p), b] = sum_t lq[(m,p), (b,t)] * m1t[(m,p), (t,b)]
        lqv = lq[:].rearrange("p (b t) -> p t b", b=BL)      # [96, 65, 4]
        dparts = []
        for bank in range(2):
            prod = wp.tile([96, 32 * BL], f32, tag="prod", name=f"prod{bank}")
            pv = prod[:].rearrange("p (t b) -> p t b", b=BL)  # [96, 32, 4]
            nc.vector.tensor_tensor(
                pv[:], lqv[:, 1 + bank * 32:1 + bank * 32 + 32, :],
                m1t[bank][:].rearrange("p (t b) -> p t b", b=BL), Alu.mult)
            dp = wp.tile([96, BL], f32, tag="dpart", name=f"dpart{bank}")
            nc.vector.tensor_reduce(
                dp[:], prod[:].rearrange("p (t b) -> p b t", b=BL),
                mybir.AxisListType.X, Alu.add)
            dparts.append(dp)
        dsum = wp.tile([96, BL], f32, tag="dsum")
        nc.vector.tensor_tensor(dsum[:], dparts[0][:], dparts[1][:], Alu.add)

        d1p = ps.tile([3, BL], f32, tag="small")
        nc.tensor.matmul(d1p[:], cview("sel96"), dsum[:], start=True, stop=True)
        d1r = wp.tile([3, BL], f32, tag="d1r")
        nc.scalar.activation(d1r[:], d1p[:], Act.Relu,
                             bias=cview("d1b3"), scale=1.0)

        op = ps.tile([BL, N], f32, tag="small")
        nc.tensor.matmul(op[:], d1r[:], cview("d2w"), start=True, stop=False)
        nc.tensor.matmul(op[:], cview("ones14"), cview("d2b"),
                         start=False, stop=True)
        out_sb = wp.tile([BL, N], f32, tag="outsb")
        nc.scalar.copy(out_sb[:], op[:])
        dma(out_d[:], out_sb[:])

    nc.compile()
    return nc


def fold_inputs(inputs):
    """Host-side weight folding. Returns the per-core-common input dict."""
    f32 = np.float32
    g = {k: np.asarray(v, f32) for k, v in inputs.items()}
    Wk, Wr, lb = g["lstm_k"], g["lstm_r"], g["lstm_b"]

    blk0 = np.arange(2 * U)            # (i, f)
    blk1 = 2 * U + np.arange(2 * U)    # (g, o)
    gsc = np.concatenate([2 * np.ones(U, f32), np.ones(U, f32)])

    sl = g["bnl_g"] / np.sqrt(g["bnl_v"] + EPS)
    tl = g["bnl_b"] - g["bnl_m"] * sl
    g1s = g["bn1_g"] / np.sqrt(g["bn1_v"] + EPS)
    d1s = g["bn1_b"] - g["bn1_m"] * g1s
    g2s = g["bn2_g"] / np.sqrt(g["bn2_v"] + EPS)
    d2s = g["bn2_b"] - g["bn2_m"] * g2s

    # structural requirements of the rank-1 GCN collapse
    assert np.abs(g["b1"]).max() == 0.0, "kernel requires b1 == 0"
    assert np.abs(d1s @ g["w2"]).max() < 1e-30, "kernel requires bn1 shift @ w2 == 0"
    assert np.abs(g["b2"]).max() == 0.0, "kernel requires b2 == 0"
    assert (g2s > 0).all(), "kernel requires positive bn2 scale"

    w2pp = (g1s[:, None] * g["w2"]) * g2s[None, :]
    D4 = g["d1_w"].reshape(T, N, K2, 3)
    constp = np.einsum("m,tnmp->p", d2s, D4)
    c1v = tl @ g["w1"]

    vals = {
        "ident": np.eye(128, dtype=f32),
        "wk0": Wk[:, blk0], "wk1": Wk[:, blk1] * gsc[None, :],
        "wr0": Wr[:, blk0], "wr1": Wr[:, blk1] * gsc[None, :],
        "w1p": sl[:, None] * g["w1"],
        "w2rep": np.repeat(w2pp, 3, axis=1),
        "d2w": g["d2_w"],
        "sel96": np.kron(np.ones((K2, 1), f32), np.eye(3, dtype=f32)),
        "d2b": g["d2_b"].reshape(1, N),
        "ones14": np.ones((1, BL), f32),
        "bias0": lb[blk0].reshape(128, 1),
        "bias1": (gsc * lb[blk1]).reshape(128, 1),
        "scale1": np.concatenate(
            [2 * np.ones(U, f32), np.ones(U, f32)]).reshape(128, 1),
        "c1": c1v.reshape(K1, 1),
        "c1n": (-c1v).reshape(K1, 1),
        "ones128": np.ones((128, 1), f32),
        "d1b3": (g["d1_b"] + constp).reshape(3, 1),
    }
    cb = np.zeros((128, BUNDLE_W), f32)
    for name, (off, rows, cols) in _BUNDLE.items():
        v = vals[name]
        assert v.shape == (rows, cols), (name, v.shape, (rows, cols))
        cb[0:rows, off:off + cols] = v
    return {"cbundle": cb, "d1w": np.ascontiguousarray(g["d1_w"])}


def _cast_d1w(arr, fast_m1):
    if fast_m1 == "bf16":
        import ml_dtypes
        return np.ascontiguousarray(arr.astype(ml_dtypes.bfloat16))
    return arr


def make_in_maps(inputs, fast_m1="f32"):
    common = fold_inputs(inputs)
    common["d1w"] = _cast_d1w(common["d1w"], fast_m1)
    x = np.asarray(inputs["x"], np.float32)
    a = np.asarray(inputs["a"], np.float32)
    in_maps = []
    for core in range(NCORE):
        m = dict(common)
        m["x_sh"] = np.ascontiguousarray(
            x[core * BL:(core + 1) * BL].reshape(BL * T, F))
        m["a_sh"] = np.ascontiguousarray(a[core * BL:(core + 1) * BL])
        in_maps.append(m)
    return in_maps


def kernel(**inputs):
    from concourse.bass_utils import run_bass_kernel_spmd

    if "module" not in _CACHE:
        _CACHE["module"] = build_module(fast_z=FAST_Z, fast_m1=FAST_M1)
    nc = _CACHE["module"]

    in_maps = make_in_maps(inputs, FAST_M1)
    res = run_bass_kernel_spmd(nc, in_maps, core_ids=list(range(NCORE)))
    out = np.concatenate([res.results[i]["out_sh"] for i in range(NCORE)], axis=0)
    return out.astype(np.float32)


# revision 24
# speedup vs baseline: 1.9919x; 1.0672x over previous
"""Trainium2 Bass kernel for nn_EndToEndCryptoModel (LSTM -> GCNx2 -> Dense).

Strategy (per-core, data-parallel over batch, 4 batches/core on 8 cores):
  * LSTM solved by Picard fixed-point iteration over the whole sequence:
    7 iterations, each fully parallel over (b, t) using big sigmoid ops
    (all 4 gates via sigmoid; tanh(y) = 2*sigmoid(2y)-1 with the 2x folded
    into weights / per-partition ACT scale), and the cell-state recurrence
    done by a single DVE tensor_tensor_scan along the time axis (batch
    chains separated by poison pad columns that reset the scan).
  * The GCN collapses algebraically: sup1 is node-independent, so
    g1 = leaky(rowsum(a) (x) s1) is rank-1 (leaky is positively homogeneous
    and b1 == 0), and the whole two-layer GCN reduces to per-(t,m) scalars
    q[t,m] and per-node weights w[n] = (a @ (a @ 1))[n].
  * Final dense layer: d1[b,p] = sum_{t,m} Lq'[b,t,m] * (w[b,:] @ D[t,:,m,p]),
    computed as 64 small matmuls with D t-slices as stationary weights into a
    [(m,p)=96, (t,b)] PSUM layout that exactly matches the layout q comes in
    (w2 columns pre-replicated x3 on the host), then DVE multiply+reduce and
    two tiny matmuls.

All heavy layout decisions are hardcoded for the fixed problem shapes.
"""

import numpy as np

B, T, N, F = 32, 64, 128, 128
U, K1, K2 = 64, 64, 32
NCORE = 8
BL = B // NCORE            # 4 batches per core
CW = BL * (T + 1)          # 260 columns, b-major with pad col at b*(T+1)
NEG = -1e30
EPS = 1e-3
SLOPE = 0.01
N_ITERS = 6

# dtype configuration for the fast matmul paths
FAST_Z = True
FAST_M1 = "f32r"

_CACHE = {}

# constant-bundle column layout: name -> (col_off, rows, cols)
_BUNDLE = {}
_off = 0
for _name, _rows, _cols in [
    ("ident", 128, 128), ("wk0", 128, 128), ("wk1", 128, 128),
    ("wr0", 64, 128), ("wr1", 64, 128), ("w1p", 64, 64), ("w2rep", 64, 96),
    ("d2w", 3, 128), ("sel96", 96, 3), ("d2b", 1, 128), ("ones14", 1, 4),
    ("bias0", 128, 1), ("bias1", 128, 1), ("scale1", 128, 1),
    ("c1", 64, 1), ("c1n", 64, 1), ("ones128", 128, 1), ("d1b3", 3, 1),
]:
    _BUNDLE[_name] = (_off, _rows, _cols)
    _off += _cols
BUNDLE_W = _off


def build_module(fast_z=False, fast_m1="f32"):
    """Build the per-core Bass/Tile module (identical SPMD program).

    fast_z: use float32r for the per-iteration z matmuls (xz + Wr@h).
    fast_m1: use float32r for the M1T (dense-layer) matmuls.
    """
    from contextlib import ExitStack
    import concourse.bacc as bacc
    import concourse.mybir as mybir
    from concourse import tile

    f32 = mybir.dt.float32
    f32r = mybir.dt.float32r
    bf16 = mybir.dt.bfloat16
    zdt = f32r if fast_z else f32
    mdt = {"f32": f32, "f32r": f32r, "bf16": bf16}[fast_m1]
    Alu = mybir.AluOpType
    Act = mybir.ActivationFunctionType

    nc = bacc.Bacc(None, target_bir_lowering=False)

    # ---------------- DRAM I/O ----------------
    x_d = nc.dram_tensor("x_sh", [BL * T, F], f32, kind="ExternalInput")
    a_d = nc.dram_tensor("a_sh", [BL, N, N], f32, kind="ExternalInput")
    cb_d = nc.dram_tensor("cbundle", [128, BUNDLE_W], f32, kind="ExternalInput")
    d1w_d = nc.dram_tensor("d1w", [T * N * K2, 3], mdt, kind="ExternalInput")
    out_d = nc.dram_tensor("out_sh", [BL, N], f32, kind="ExternalOutput")

    with tile.TileContext(nc) as tc, ExitStack() as ctx:
        cp = ctx.enter_context(tc.tile_pool(name="const", bufs=1))
        wp = ctx.enter_context(tc.tile_pool(name="work", bufs=2))
        pz = ctx.enter_context(tc.tile_pool(name="pz", bufs=1, space="PSUM"))
        pm = ctx.enter_context(tc.tile_pool(name="pm", bufs=2, space="PSUM"))
        pt = ctx.enter_context(tc.tile_pool(name="pt", bufs=2, space="PSUM"))
        ps = ctx.enter_context(tc.tile_pool(name="ps", bufs=2, space="PSUM"))

        dma = nc.sync.dma_start

        # ---- DMAs: constants bundle + x first, then a; big D last ----
        cb = cp.tile([128, BUNDLE_W], f32, tag="cb")
        dma(cb[:], cb_d[:])

        def cview(name):
            off, rows, cols = _BUNDLE[name]
            return cb[0:rows, off:off + cols]

        x2 = wp.tile([128, 256], f32, tag="x2")
        dma(x2[:].rearrange("p (i f) -> p i f", i=2),
            x_d[:].rearrange("(i p) f -> p i f", i=2))

        a_all = wp.tile([128, BL * N], f32, tag="a_all")
        dma(a_all[:].rearrange("p (b n) -> p b n", b=BL),
            a_d[:].rearrange("b p n -> p b n"))

        D_sb = cp.tile([128, T * K2 * 3], mdt, tag="Dsb")
        d1w_view = d1w_d[:].rearrange("(t n m) p -> n t (m p)", t=T, n=N, m=K2)
        dma(D_sb[:].rearrange("n (t mp) -> n t mp", t=T), d1w_view)

        # preload the sigmoid table set while DMAs stream in
        warm = cp.tile([1, 1], f32, tag="warm")
        nc.scalar.activation(warm[:], cb[0:1, 0:1], Act.Sigmoid)

        ident = cview("ident")
        if fast_z:
            identr = cp.tile([128, 128], f32r, tag="identr")
            nc.vector.tensor_copy(identr[:], ident)
            ident_z = identr[:]
        else:
            ident_z = ident

        # ---- x transpose + xz precompute ----
        xt_sb = []
        for i in range(2):
            tp = pt.tile([128, 128], f32, tag="tp")
            nc.tensor.transpose(tp[:], x2[:, i * 128:(i + 1) * 128], ident)
            xt = cp.tile([128, 128], f32, tag=f"xt{i}")
            nc.vector.tensor_copy(xt[:], tp[:])
            xt_sb.append(xt)

        # xz[blk] = Wk_blk.T @ xT  scattered to b-major pad layout
        # [128, 260], written directly into iteration 0's z psum banks.
        # SBUF copies (used by iterations 1+) evacuate on ACT off-path.
        xzt = []
        z_it0 = []
        for blk in (1, 0):
            wk = cview("wk0" if blk == 0 else "wk1")
            xzp = pz.tile([128, CW], f32, tag=f"z{blk}", name=f"z{blk}_xz")
            xzp3 = xzp[:].rearrange("p (b t) -> p b t", b=BL)
            for b in range(BL):
                i, bl = divmod(b, 2)
                nc.tensor.matmul(
                    xzp[:, b * (T + 1) + 1:b * (T + 1) + 1 + T],
                    wk[:], xt_sb[i][:, bl * T:(bl + 1) * T],
                    start=True, stop=True,
                )
            nc.vector.memset(xzp3[:, :, 0:1], NEG)
            xz_sb = cp.tile([128, CW], zdt, tag=f"xzt{blk}")
            z_it0.append(xzp)
            xzt.append(xz_sb)
        z_it0 = {1: z_it0[0], 0: z_it0[1]}
        xzt = [xzt[1], xzt[0]]

        def evac_xz():
            for blk in range(2):
                xz3 = xzt[blk][:].rearrange("p (b t) -> p b t", b=BL)
                zp3 = z_it0[blk][:].rearrange("p (b t) -> p b t", b=BL)
                nc.scalar.copy(xz3[:, :, 1:T + 1], zp3[:, :, 1:T + 1])
                nc.vector.memset(xz3[:, :, 0:1].bitcast(f32), NEG)

        # wcols [128 (n), 4 (b)] filled by A-prep inside iteration 0
        wcols = cp.tile([128, BL], mdt, tag="wcols")
        ones128 = cview("ones128")

        # M1T psum: [(m,p)=96, (t,b)=256], filled by 64 per-t matmuls
        # (lhsT = D t-slice) interleaved into the LSTM iterations
        m1t = pm.tile([96, T * BL], f32, tag="m1t")

        # ---- LSTM Picard iterations ----
        bias0, bias1, scale1 = cview("bias0"), cview("bias1"), cview("scale1")
        wr0z = cp.tile([U, 128], zdt, tag="wr0z")
        nc.vector.tensor_copy(wr0z[:], cview("wr0"))
        wr1z = cp.tile([U, 128], zdt, tag="wr1z")
        nc.vector.tensor_copy(wr1z[:], cview("wr1"))

        h = None
        m1_sched = {1: 13, 2: 13, 3: 13, 4: 13}
        m1_done = 0

        for it in range(N_ITERS):
            # z1 (g,o block) first: its sigmoid leads the critical chain
            zp = {}
            if it == 0:
                zp = z_it0
            else:
                for blk, wr, xz_sb in ((1, wr1z, xzt[1]), (0, wr0z, xzt[0])):
                    z = pz.tile([128, CW], f32, tag=f"z{blk}",
                                name=f"z{blk}_{it}")
                    nc.tensor.matmul(z[:], ident_z, xz_sb[:],
                                     start=True, stop=False)
                    nc.tensor.matmul(z[:], wr[:], h[:, 0:CW],
                                     start=False, stop=True)
                    zp[blk] = z
            # s1 = sigmoid(scale1*z_go + b_go'): Sg'=sig(2 z_g) rows 0:64,
            # So rows 64:128
            s1 = wp.tile([128, CW], f32, tag="s1")
            nc.scalar.activation(s1[:], zp[1][:], Act.Sigmoid,
                                 bias=bias1, scale=1.0)
            # s0 = sigmoid(z_if + b_if): Si rows 0:64, Sf rows 64:128
            s0 = wp.tile([128, CW], f32, tag="s0")
            nc.scalar.activation(s0[:], zp[0][:], Act.Sigmoid,
                                 bias=bias0, scale=1.0)
            # g2 = tanh(z_g) = 2*Sg' - 1  (runs on DVE while s0 is on ACT)
            g2 = wp.tile([U, CW], f32, tag="g2")
            nc.vector.tensor_scalar(g2[:], s1[0:U], 2.0, 1.0,
                                    Alu.mult, Alu.subtract)
            # v = i*g = Si*g2, written at base partition 64 so the scan's
            # two inputs (Sf, v) share a base
            v = wp.tile([128, CW], f32, tag="v")
            nc.vector.tensor_tensor(v[U:128], s0[0:U], g2[:], Alu.mult)
            c = wp.tile([128, CW], f32, tag="c")
            nc.vector.tensor_tensor_scan(
                c[U:128], s0[U:128], v[U:128], 0.0, Alu.mult, Alu.add)
            th = wp.tile([128, CW], f32, tag="th")
            nc.scalar.activation(th[U:128], c[U:128], Act.Tanh,
                                 bias=0.0, scale=1.0)
            h = wp.tile([U, CW + 1], zdt, tag="h")
            nc.vector.tensor_tensor(h[:, 1:CW + 1], s1[U:128], th[U:128],
                                    Alu.mult)
            nc.vector.memset(h[:, 0:1].bitcast(f32), 0.0)

            if it == 0:
                # xz evac for iterations 1+ (ACT; off the critical chain)
                evac_xz()
                # A-prep in iteration 0's engine gaps (copies on ACT):
                # AT, r = A@1, w = A@r -> wcols
                for b in range(BL):
                    tp = pt.tile([128, 128], f32, tag="tp")
                    nc.tensor.transpose(tp[:], a_all[:, b * N:(b + 1) * N],
                                        ident)
                    at_sb = wp.tile([128, N], f32, tag="atsb")
                    nc.scalar.copy(at_sb[:], tp[:])
                    rp = ps.tile([128, 1], f32, tag="small")
                    nc.tensor.matmul(rp[:], at_sb[:], ones128,
                                     start=True, stop=True)
                    r_sb = wp.tile([128, 1], f32, tag="rsb")
                    nc.scalar.copy(r_sb[:], rp[:])
                    wpm = ps.tile([128, 1], f32, tag="small")
                    nc.tensor.matmul(wpm[:], at_sb[:], r_sb[:],
                                     start=True, stop=True)
                    nc.scalar.copy(wcols[:, b:b + 1], wpm[:])

            # M1T matmuls (need D + wcols only), hidden inside iterations
            for t in range(m1_done, m1_done + m1_sched.get(it, 0)):
                nc.tensor.matmul(
                    m1t[:, t * BL:(t + 1) * BL],
                    D_sb[:, t * 96:(t + 1) * 96], wcols[:],
                    start=True, stop=True,
                )
            m1_done += m1_sched.get(it, 0)

        # ---- GCN tail ----
        if fast_z:
            w1pz = cp.tile([U, K1], zdt, tag="w1pz")
            nc.vector.tensor_copy(w1pz[:], cview("w1p"))
            w1p_use = w1pz[:]
        else:
            w1p_use = cview("w1p")
        s1p = pt.tile([K1, CW], f32, tag="tp")
        nc.tensor.matmul(s1p[:], w1p_use, h[:, 1:CW + 1],
                         start=True, stop=True)
        # leaky(y) = y + (1-slope)*relu(-y), y = s1 + c1
        rn1 = wp.tile([K1, CW], f32, tag="rn1")
        nc.scalar.activation(rn1[:], s1p[:], Act.Relu,
                             bias=cview("c1n"), scale=-1.0)
        L1a = wp.tile([K1, CW], f32, tag="L1a")
        nc.vector.scalar_tensor_tensor(
            L1a[:], rn1[:], 1.0 - SLOPE, s1p[:], Alu.mult, Alu.add)
        L1 = wp.tile([K1, CW], zdt, tag="L1")
        nc.vector.tensor_scalar_add(L1[:], L1a[:], cview("c1"))
        if fast_z:
            w2z = cp.tile([K1, 96], zdt, tag="w2z")
            nc.vector.tensor_copy(w2z[:], cview("w2rep"))
            w2_use = w2z[:]
        else:
            w2_use = cview("w2rep")
        qp = pt.tile([96, CW], f32, tag="tp")
        nc.tensor.matmul(qp[:], w2_use, L1[:], start=True, stop=True)
        rn2 = wp.tile([96, CW], f32, tag="rn2")
        nc.scalar.activation(rn2[:], qp[:], Act.Relu, bias=0.0, scale=-1.0)
        lq = wp.tile([96, CW], f32, tag="lq")
        nc.vector.scalar_tensor_tensor(
            lq[:], rn2[:], 1.0 - SLOPE, qp[:], Alu.mult, Alu.add)

        # last M1T matmuls traced here so they overlap the Lq chain above
        for t in range(m1_done, T):
            nc.tensor.matmul(
                m1t[:, t * BL:(t + 1) * BL],
                D_sb[:, t * 96:(t + 1) * 96], wcols[:],
                start=True, stop=True,
            )

        # dsum[(m,p), b] = sum_t lq[(m,p), (b,t)] * m1t[(m,p), (t,b)]
        lqv = lq[:].rearrange("p (b t) -> p t b", b=BL)      # [96, 65, 4]
        prod = wp.tile([96, T * BL], f32, tag="prod")
        pv = prod[:].rearrange("p (t b) -> p t b", b=BL)      # [96, 64, 4]
        nc.vector.tensor_tensor(
            pv[:], lqv[:, 1:T + 1, :],
            m1t[:].rearrange("p (t b) -> p t b", b=BL), Alu.mult)
        dsum = wp.tile([96, BL], f32, tag="dsum")
        nc.vector.tensor_reduce(
            dsum[:], prod[:].rearrange("p (t b) -> p b t", b=BL),
            mybir.AxisListType.X, Alu.add)

        d1p = ps.tile([3, BL], f32, tag="small")
        nc.tensor.matmul(d1p[:], cview("sel96"), dsum[:], start=True, stop=True)
        d1r = wp.tile([3, BL], f32, tag="d1r")
        nc.scalar.activation(d1r[:], d1p[:], Act.Relu,
                             bias=cview("d1b3"), scale=1.0)

        op = ps.tile([BL, N], f32, tag="small")
        nc.tensor.matmul(op[:], d1r[:], cview("d2w"), start=True, stop=False)
        nc.tensor.matmul(op[:], cview("ones14"), cview("d2b"),
                         start=False, stop=True)
        out_sb = wp.tile([BL, N], f32, tag="outsb")
        nc.scalar.copy(out_sb[:], op[:])
        dma(out_d[:], out_sb[:])

    nc.compile()
    return nc


def fold_inputs(inputs):
    """Host-side weight folding. Returns the per-core-common input dict."""
    f32 = np.float32
    g = {k: np.asarray(v, f32) for k, v in inputs.items()}
    Wk, Wr, lb = g["lstm_k"], g["lstm_r"], g["lstm_b"]

    blk0 = np.arange(2 * U)            # (i, f)
    blk1 = 2 * U + np.arange(2 * U)    # (g, o)
    gsc = np.concatenate([2 * np.ones(U, f32), np.ones(U, f32)])

    sl = g["bnl_g"] / np.sqrt(g["bnl_v"] + EPS)
    tl = g["bnl_b"] - g["bnl_m"] * sl
    g1s = g["bn1_g"] / np.sqrt(g["bn1_v"] + EPS)
    d1s = g["bn1_b"] - g["bn1_m"] * g1s
    g2s = g["bn2_g"] / np.sqrt(g["bn2_v"] + EPS)
    d2s = g["bn2_b"] - g["bn2_m"] * g2s

    # structural requirements of the rank-1 GCN collapse
    assert np.abs(g["b1"]).max() == 0.0, "kernel requires b1 == 0"
    assert np.abs(d1s @ g["w2"]).max() < 1e-30, "kernel requires bn1 shift @ w2 == 0"
    assert np.abs(g["b2"]).max() == 0.0, "kernel requires b2 == 0"
    assert (g2s > 0).all(), "kernel requires positive bn2 scale"

    w2pp = (g1s[:, None] * g["w2"]) * g2s[None, :]
    D4 = g["d1_w"].reshape(T, N, K2, 3)
    constp = np.einsum("m,tnmp->p", d2s, D4)
    c1v = tl @ g["w1"]

    vals = {
        "ident": np.eye(128, dtype=f32),
        "wk0": Wk[:, blk0], "wk1": Wk[:, blk1] * gsc[None, :],
        "wr0": Wr[:, blk0], "wr1": Wr[:, blk1] * gsc[None, :],
        "w1p": sl[:, None] * g["w1"],
        "w2rep": np.repeat(w2pp, 3, axis=1),
        "d2w": g["d2_w"],
        "sel96": np.kron(np.ones((K2, 1), f32), np.eye(3, dtype=f32)),
        "d2b": g["d2_b"].reshape(1, N),
        "ones14": np.ones((1, BL), f32),
        "bias0": lb[blk0].reshape(128, 1),
        "bias1": (gsc * lb[blk1]).reshape(128, 1),
        "scale1": np.concatenate(
            [2 * np.ones(U, f32), np.ones(U, f32)]).reshape(128, 1),
        "c1": c1v.reshape(K1, 1),
        "c1n": (-c1v).reshape(K1, 1),
        "ones128": np.ones((128, 1), f32),
        "d1b3": (g["d1_b"] + constp).reshape(3, 1),
    }
    cb = np.zeros((128, BUNDLE_W), f32)
    for name, (off, rows, cols) in _BUNDLE.items():
        v = vals[name]
        assert v.shape == (rows, cols), (name, v.shape, (rows, cols))
        cb[0:rows, off:off + cols] = v
    return {"cbundle": cb, "d1w": np.ascontiguousarray(g["d1_w"])}


def _cast_d1w(arr, fast_m1):
    if fast_m1 == "bf16":
        import ml_dtypes
        return np.ascontiguousarray(arr.astype(ml_dtypes.bfloat16))
    return arr


def make_in_maps(inputs, fast_m1="f32"):
    common = fold_inputs(inputs)
    common["d1w"] = _cast_d1w(common["d1w"], fast_m1)
    x = np.asarray(inputs["x"], np.float32)
    a = np.asarray(inputs["a"], np.float32)
    in_maps = []
    for core in range(NCORE):
        m = dict(common)
        m["x_sh"] = np.ascontiguousarray(
            x[core * BL:(core + 1) * BL].reshape(BL * T, F))
        m["a_sh"] = np.ascontiguousarray(a[core * BL:(core + 1) * BL])
        in_maps.append(m)
    return in_maps


def kernel(**inputs):
    from concourse.bass_utils import run_bass_kernel_spmd

    if "module" not in _CACHE:
        _CACHE["module"] = build_module(fast_z=FAST_Z, fast_m1=FAST_M1)
    nc = _CACHE["module"]

    in_maps = make_in_maps(inputs, FAST_M1)
    res = run_bass_kernel_spmd(nc, in_maps, core_ids=list(range(NCORE)))
    out = np.concatenate([res.results[i]["out_sh"] for i in range(NCORE)], axis=0)
    return out.astype(np.float32)
